# revision 1
# baseline (speedup 1.0000x reference)
"""Trainium2 Bass kernel for 2-layer ChebConv (K=3) on a 200k-node/3.2M-edge graph.

Math (PyG ChebConv, sym norm, lambda_max=2 => L_hat = -D^-1/2 A D^-1/2):
  With dis = deg^-1/2 (0 for isolated), L z = -dis * (A (dis * z)), and A commutes
  with right-multiplication by weight matrices.  One layer (K=3):
    out = x@(W0-W2) + Tx1@W1 + 2*(-dis)*(A q),   Tx1 = -dis*(A(dis*x)),
    q   = dis*(Tx1@W2)
  Layer 2 identical with h = relu(out1 + b1), projections at width C=2 first.

Mapping: nodes are LPT-packed into per-core dest tiles of 128 slots; all node
tables are stored in one permuted layout [GS, *] replicated per core; per tile,
edges are bucketed by int16 table chunk (32768 rows) and gathered with the Ant
dma_gather instruction (4 SWDGE queues, trailing -1 indices trim pad slots at
zero bandwidth); segment-sums are selector-matrix matmuls accumulated in PSUM;
the [N,H] intermediate q (and the small layer-2 tables) are exchanged with
AllGather collectives inside one SPMD NEFF on 8 cores.  The host does index
work only (bucketing, packing, degree counts, permutation).
"""
import os
import heapq
import numpy as np

import concourse.bass as bass
import concourse.bacc as bacc
import concourse.tile as tile
import concourse.bass_utils as bass_utils
from concourse import mybir
from concourse.masks import make_identity

P = 128          # partitions / edges per block / dest slots per tile
NCORES = 8
CHUNK = 32768    # int16-addressable table rows per gather call
F32 = mybir.dt.float32
I16 = mybir.dt.int16
F_TAB = None     # set per problem: padded table width (mult of 64 f32)


def _pad_width(w):
    return ((w + 63) // 64) * 64      # 256-byte multiple in f32


# ----------------------------------------------------------------------------
# Host-side preprocessing (index work only)
# ----------------------------------------------------------------------------

def preprocess(x, edge_index, bpt_cap=None, gmode="ind"):
    N, F = x.shape
    row = np.asarray(edge_index[0]).astype(np.int64)
    col = np.asarray(edge_index[1]).astype(np.int64)
    E = row.shape[0]

    deg = np.bincount(row, minlength=N)
    dis = np.where(deg > 0, 1.0 / np.sqrt(np.maximum(deg, 1)), 0.0).astype(np.float32)

    npc = (N + NCORES - 1) // NCORES
    core_of = np.minimum(np.arange(N) // npc, NCORES - 1)

    avg_deg = E / max(N, 1)
    if bpt_cap is None:
        bpt_cap = max(2, int(np.ceil(P * avg_deg / P * 1.07)))  # edge blocks per tile
    cap = bpt_cap * P

    # --- per-core LPT packing of nodes into tiles (<=P nodes, <=cap degree) ---
    tile_of = np.zeros(N, dtype=np.int64)
    slot_of = np.zeros(N, dtype=np.int64)
    T = 0
    for c in range(NCORES):
        nodes = np.where(core_of == c)[0]
        degs = deg[nodes]
        total = int(degs.sum())
        Tc = max(int(np.ceil(len(nodes) / P)), int(np.ceil(total / (cap * 0.97))))
        while True:
            order = np.argsort(-degs, kind="stable")
            heap = [(0, 0, t) for t in range(Tc)]
            heapq.heapify(heap)
            ok = True
            tl = np.empty(len(nodes), dtype=np.int64)
            sl = np.empty(len(nodes), dtype=np.int64)
            for i in order:
                d = int(degs[i])
                spill = []
                while True:
                    if not heap:
                        ok = False
                        break
                    load, cnt, t = heapq.heappop(heap)
                    if cnt < P and load + d <= cap:
                        tl[i], sl[i] = t, cnt
                        if cnt + 1 < P:
                            heapq.heappush(heap, (load + d, cnt + 1, t))
                        break
                    elif cnt < P:
                        spill.append((load, cnt, t))
                for s in spill:
                    heapq.heappush(heap, s)
                if not ok:
                    break
            if ok:
                break
            Tc += max(1, Tc // 50)
        tile_of[nodes] = tl
        slot_of[nodes] = sl
        T = max(T, Tc)

    SLOTS = T * P
    GS = NCORES * SLOTS
    NCH = (GS + CHUNK - 1) // CHUNK if gmode == "ant" else 1
    perm = (core_of * SLOTS + tile_of * P + slot_of).astype(np.int64)

    # --- edges sorted by (core, tile, chunk of permuted source) ---
    colp = perm[col]
    ec = core_of[row]
    et = tile_of[row]
    chunk = (colp >> 15) if gmode == "ant" else np.zeros(E, dtype=np.int64)
    key = ((ec * T + et) * NCH + chunk)
    eorder = np.argsort(key, kind="stable")
    ks = key[eorder]
    colp_s = colp[eorder]
    dslot_s = slot_of[row[eorder]]
    dcol_s = dis[col[eorder]]

    # bucket counts [NCORES*T*NCH]
    nbuckets = NCORES * T * NCH
    bc = np.bincount(ks, minlength=nbuckets).reshape(NCORES, T, NCH)
    # shared schedule: blocks per (tile, chunk) = max over cores
    blocks_tc = np.ceil(bc.max(axis=0) / P).astype(np.int64)       # [T, NCH]
    nblk_t = blocks_tc.sum(axis=1)                                  # [T]
    NBLK = int(nblk_t.sum())                                        # total block-slots
    # per-(tile,chunk) offsets in block-slot space
    bs_off_tc = np.zeros((T, NCH), dtype=np.int64)
    flat = blocks_tc.reshape(-1)
    bs_off_tc.reshape(-1)[1:] = np.cumsum(flat)[:-1]

    # --- per-core edge arrays ---
    # rank of each edge within its (core,tile,chunk) bucket
    start_of = np.zeros(nbuckets + 1, dtype=np.int64)
    np.cumsum(bc.reshape(-1), out=start_of[1:])
    rank = np.arange(E) - start_of[ks]
    lane = rank % P
    blk = rank // P
    bs = bs_off_tc.reshape(-1)[ks % (T * NCH)] + blk               # block-slot per edge

    # selector metadata [NCORES, P, NBLK]
    dslot_arr = np.full((NCORES, P, NBLK), 999.0, dtype=np.float32)
    discol_arr = np.zeros((NCORES, P, NBLK), dtype=np.float32)
    dslot_arr[ec[eorder], lane, bs] = dslot_s.astype(np.float32)
    discol_arr[ec[eorder], lane, bs] = dcol_s

    # gather idx arrays, wrapped-16 and replicated to 128 partitions.
    # per bucket: capacity blocks_tc*P positions; columns = capacity*P/16
    IDXCOLS = NBLK * (P // 16)
    idx16 = np.full((NCORES, 16, IDXCOLS), -1, dtype=np.int16)
    pos_in_bucket = rank                                            # 0..n-1 (trailing pads)
    col_off = (bs_off_tc.reshape(-1)[ks % (T * NCH)] * (P // 16)
               + pos_in_bucket // 16)
    if gmode == "ant":
        idx16[ec[eorder], pos_in_bucket % 16, col_off] = (colp_s & 32767).astype(np.int16)
    idx_rep = np.tile(idx16, (1, 8, 1))                             # [NCORES, 128, IDXCOLS]
    col32 = np.zeros((NCORES, P, NBLK), dtype=np.int32)
    col32[ec[eorder], lane, bs] = colp_s.astype(np.int32)

    dis_slot = np.zeros((NCORES, P, T), dtype=np.float32)
    dis_slot[core_of, slot_of, tile_of] = dis

    # per-tile call schedule [(chunk, nblocks, idxcol_off, bs_off), ...]
    sched = []
    for t in range(T):
        calls = []
        for ch in range(NCH):
            nb = int(blocks_tc[t, ch])
            if nb == 0:
                continue
            calls.append((ch, nb, int(bs_off_tc[t, ch]) * (P // 16),
                          int(bs_off_tc[t, ch])))
        sched.append(calls)

    meta = dict(N=N, F=F, E=E, T=T, SLOTS=SLOTS, GS=GS, NCH=NCH, NBLK=NBLK,
                MAXB=int(blocks_tc.max()), perm=perm, dis=dis, core_of=core_of,
                sched=sched, pad_ratio=NBLK * P * NCORES / E - 1)
    meta["gmode"] = gmode
    arrays = dict(dslot=dslot_arr, discol=discol_arr, idx=idx_rep,
                  col32=col32, dis_slot=dis_slot)
    return meta, arrays


# ----------------------------------------------------------------------------
# Bass kernel builder (SPMD; shared schedule, per-core data)
# ----------------------------------------------------------------------------

def build_kernel(tc, io, cfg):
    nc = tc.nc
    F, H, C = cfg["F"], cfg["H"], cfg["C"]
    T, SLOTS, GS = cfg["T"], cfg["SLOTS"], cfg["GS"]
    NBLK, MAXB, sched = cfg["NBLK"], cfg["MAXB"], cfg["sched"]
    FT = _pad_width(F)            # x table width
    QT = _pad_width(H)            # q table width
    CT = _pad_width(2 * C)        # c table width
    QT2 = _pad_width(C)           # q2 table width
    C2 = 2 * C
    IDXC = P // 16
    fchunks = [(0, min(FT, P))] + ([(P, FT)] if FT > P else [])
    NQ = cfg.get("NQ", 4)

    ident_pool = tc.alloc_tile_pool(name="ident", bufs=1)
    const = tc.alloc_tile_pool(name="const", bufs=1)
    gpool = tc.alloc_tile_pool(name="gather", bufs=3)
    selp = tc.alloc_tile_pool(name="sel", bufs=2)
    sbw = tc.alloc_tile_pool(name="work", bufs=3)
    pprop = tc.alloc_tile_pool(name="pprop", bufs=2, space="PSUM")
    paux = tc.alloc_tile_pool(name="paux", bufs=2, space="PSUM")
    paux2 = tc.alloc_tile_pool(name="paux2", bufs=4, space="PSUM")

    ident = ident_pool.tile([P, P], F32, tag="ident")
    make_identity(nc, ident[:])

    GMODE = cfg.get("GMODE", "ind")
    idx_sb = const.tile([P, NBLK * IDXC], I16, tag="idx")
    col32_sb = const.tile([P, NBLK], mybir.dt.int32, tag="col32")
    dslot_sb = const.tile([P, NBLK], F32, tag="dslot")
    discol_sb = const.tile([P, NBLK], F32, tag="discol")
    if GMODE == "ant":
        nc.sync.dma_start(out=idx_sb[:], in_=io["idx"][:])
    else:
        nc.sync.dma_start(out=col32_sb[:], in_=io["col32"][:])
    nc.sync.dma_start(out=dslot_sb[:], in_=io["dslot"][:])
    nc.sync.dma_start(out=discol_sb[:], in_=io["discol"][:])

    sneg = const.tile([P, T], F32, tag="sneg")
    sdis = const.tile([P, T], F32, tag="sdis")
    sm2 = const.tile([P, T], F32, tag="sm2")
    sneg2 = const.tile([P, T], F32, tag="sneg2")
    nc.sync.dma_start(out=sneg[:], in_=io["sneg"][:])
    nc.sync.dma_start(out=sdis[:], in_=io["sdis"][:])
    nc.sync.dma_start(out=sm2[:], in_=io["sm2"][:])
    nc.sync.dma_start(out=sneg2[:], in_=io["sneg2"][:])

    iota_sb = const.tile([P, P], F32, tag="iota")
    nc.sync.dma_start(out=iota_sb[:], in_=io["iota"][:])
    b1_sb = const.tile([P, H], F32, tag="b1")
    nc.sync.dma_start(out=b1_sb[:], in_=io["b1row"][:])
    b2_sb = const.tile([P, C], F32, tag="b2")
    nc.sync.dma_start(out=b2_sb[:], in_=io["b2row"][:])

    w1A = const.tile([fchunks[0][1], 3 * H], F32, tag="w1A")
    nc.sync.dma_start(out=w1A[:], in_=io["w1cat"][0:fchunks[0][1], :])
    w1B = None
    if len(fchunks) > 1:
        w1B = const.tile([FT - P, 3 * H], F32, tag="w1B")
        nc.sync.dma_start(out=w1B[:], in_=io["w1cat"][P:FT, :])
    vall = const.tile([H, 3 * C], F32, tag="vall")
    nc.sync.dma_start(out=vall[:], in_=io["vall"][:])

    q_shard, q_full = io["q_shard"], io["q_full"]
    c_shard, c_full = io["c_shard"], io["c_full"]
    q2_shard, q2_full = io["q2_shard"], io["q2_full"]
    partial1, hT_st, ac1_st = io["partial1"], io["hT_st"], io["ac1_st"]

    state = dict(q=0, first=3 * len([0]))
    first_gx = [3]   # memset the first `bufs` gather tiles (NaN hygiene)

    def gather_tile(table_ap, t, width, per_tile, scaled):
        """Gather all buckets of tile t, selector-matmul into PSUM, call per_tile."""
        calls = sched[t]
        nblk_t = sum(nb for _, nb, _, _ in calls)
        bs0 = calls[0][3]
        # selector for the tile's full block range
        sel = selp.tile([P, MAXBT * P], F32, tag="sel")
        sel3 = sel[:, 0:nblk_t * P].rearrange("p (j d) -> p j d", d=P)
        dsl = dslot_sb[:, bs0:bs0 + nblk_t].unsqueeze(2).to_broadcast([P, nblk_t, P])
        iot = iota_sb[:].unsqueeze(1).to_broadcast([P, nblk_t, P])
        nc.vector.tensor_tensor(out=sel3, in0=dsl, in1=iot, op=mybir.AluOpType.is_equal)
        if scaled:
            dcl = discol_sb[:, bs0:bs0 + nblk_t].unsqueeze(2).to_broadcast([P, nblk_t, P])
            nc.vector.tensor_tensor(out=sel3, in0=sel3, in1=dcl, op=mybir.AluOpType.mult)
        ps = pprop.tile([P, width], F32, space="PSUM", tag="ps")
        j = 0
        for (ch, nb, ic_off, bs_off) in calls:
            gx = gpool.tile([P, MAXB * width], F32, tag="gx")
            if first_gx[0] > 0:
                first_gx[0] -= 1
                nc.vector.memset(gx[:], 0.0)
            c0 = ch * CHUNK
            c1 = min(c0 + CHUNK, GS)
            if GMODE == "ant":
                nc.gpsimd.dma_gather(
                    out_ap=gx[:, 0:nb * width].rearrange("p (k w) -> p k w", w=width),
                    in_ap=table_ap[c0:c1, :],
                    idxs_ap=idx_sb[:, ic_off:ic_off + nb * IDXC],
                    num_idxs=nb * P, num_idxs_reg=nb * P, elem_size=width,
                    queue_num=state["q"] % NQ)
                state["q"] += 1
            else:
                for b in range(nb):
                    nc.gpsimd.indirect_dma_start(
                        out=gx[:, b * width:(b + 1) * width],
                        out_offset=None, in_=table_ap,
                        in_offset=bass.IndirectOffsetOnAxis(
                            ap=col32_sb[:, bs_off + b:bs_off + b + 1], axis=0))
            for b in range(nb):
                nc.tensor.matmul(
                    out=ps[:],
                    lhsT=sel[:, (bs_off - bs0 + b) * P:(bs_off - bs0 + b + 1) * P],
                    rhs=gx[:, b * width:(b + 1) * width],
                    start=(j == 0), stop=(j == nblk_t - 1))
                j += 1
        per_tile(t, ps)

    MAXBT = max(sum(nb for _, nb, _, _ in calls) for calls in sched)

    # ---------------- pass 1 ----------------
    def pass1_tile(t, ps):
        rs = slice(t * P, (t + 1) * P)
        tx1 = sbw.tile([P, FT], F32, tag="tx1")
        nc.scalar.activation(out=tx1[:], in_=ps[:],
                             func=mybir.ActivationFunctionType.Copy,
                             scale=sneg[:, t:t + 1])
        txT = []
        for k, (a, b) in enumerate(fchunks):
            w = b - a
            pt = paux.tile([P, P], F32, space="PSUM", tag="ptr")
            nc.tensor.transpose(out=pt[0:w, :], in_=tx1[:, a:b], identity=ident[:])
            st = sbw.tile([P, P], F32, tag=f"txT{k}")
            nc.scalar.activation(out=st[0:w, :], in_=pt[0:w, :],
                                 func=mybir.ActivationFunctionType.Copy)
            txT.append((st, w))
        wch = [w1A] + ([w1B] if w1B is not None else [])
        pq = paux2.tile([P, H], F32, space="PSUM", tag="pa")
        for k, (st, w) in enumerate(txT):
            nc.tensor.matmul(out=pq[:], lhsT=st[0:w, :], rhs=wch[k][:, 2 * H:3 * H],
                             start=(k == 0), stop=(k == len(txT) - 1))
        qt = sbw.tile([P, QT], F32, tag="qt")
        if QT > H:
            nc.vector.memset(qt[:], 0.0)
        nc.scalar.activation(out=qt[:, 0:H], in_=pq[:],
                             func=mybir.ActivationFunctionType.Copy,
                             scale=sdis[:, t:t + 1])
        nc.sync.dma_start(out=q_shard[rs, :], in_=qt[:])
        pp = paux2.tile([P, H], F32, space="PSUM", tag="pa")
        first = True
        for k, (a, b) in enumerate(fchunks):
            w = b - a
            xt = sbw.tile([P, P], F32, tag=f"xT{k}")
            nc.sync.dma_start(out=xt[0:w, :], in_=io["xT_s"][a:b, t * P:(t + 1) * P])
            nc.tensor.matmul(out=pp[:], lhsT=xt[0:w, :], rhs=wch[k][:, 0:H],
                             start=first, stop=False)
            first = False
        for k, (st, w) in enumerate(txT):
            nc.tensor.matmul(out=pp[:], lhsT=st[0:w, :], rhs=wch[k][:, H:2 * H],
                             start=False, stop=(k == len(txT) - 1))
        p1 = sbw.tile([P, H], F32, tag="p1")
        nc.vector.tensor_tensor(out=p1[:], in0=pp[:], in1=b1_sb[:],
                                op=mybir.AluOpType.add)
        nc.sync.dma_start(out=partial1[rs, :], in_=p1[:])

    for t in range(T):
        gather_tile(io["x_perm"][:], t, FT, pass1_tile, True)

    nc.gpsimd.collective_compute(
        "AllGather", mybir.AluOpType.bypass,
        replica_groups=[list(range(NCORES))],
        ins=[q_shard[:].opt()], outs=[q_full[:].opt()])

    # ---------------- pass 2 ----------------
    def pass2_tile(t, ps):
        rs = slice(t * P, (t + 1) * P)
        p1 = sbw.tile([P, H], F32, tag="p1b")
        nc.sync.dma_start(out=p1[:], in_=partial1[rs, :])
        hp = sbw.tile([P, H], F32, tag="hp")
        nc.vector.tensor_tensor(out=hp[:], in0=ps[:, 0:H],
                                in1=sm2[:, t:t + 1].to_broadcast([P, H]),
                                op=mybir.AluOpType.mult)
        ht = sbw.tile([P, H], F32, tag="ht")
        nc.vector.tensor_tensor(out=ht[:], in0=hp[:], in1=p1[:], op=mybir.AluOpType.add)
        nc.scalar.activation(out=ht[:], in_=ht[:], func=mybir.ActivationFunctionType.Relu)
        pt = paux.tile([P, P], F32, space="PSUM", tag="ptr")
        nc.tensor.transpose(out=pt[0:H, :], in_=ht[:, 0:H], identity=ident[:])
        hT = sbw.tile([H, P], F32, tag="hT")
        nc.scalar.activation(out=hT[:], in_=pt[0:H, :],
                             func=mybir.ActivationFunctionType.Copy)
        nc.sync.dma_start(out=hT_st[:, t * P:(t + 1) * P], in_=hT[:])
        pc = paux2.tile([P, CT], F32, space="PSUM", tag="pa")
        nc.tensor.matmul(out=pc[:, 0:C2], lhsT=hT[:], rhs=vall[:, C:3 * C],
                         start=True, stop=True)
        ct = sbw.tile([P, CT], F32, tag="ct")
        nc.vector.memset(ct[:], 0.0)
        nc.vector.tensor_tensor(out=ct[:, 0:C2], in0=pc[:, 0:C2],
                                in1=sdis[:, t:t + 1].to_broadcast([P, C2]),
                                op=mybir.AluOpType.mult)
        nc.sync.dma_start(out=c_shard[rs, :], in_=ct[:])

    for t in range(T):
        gather_tile(q_full[:], t, QT, pass2_tile, False)

    nc.gpsimd.collective_compute(
        "AllGather", mybir.AluOpType.bypass,
        replica_groups=[list(range(NCORES))],
        ins=[c_shard[:].opt()], outs=[c_full[:].opt()])

    # ---------------- pass 3 ----------------
    def pass3_tile(t, ps):
        rs = slice(t * P, (t + 1) * P)
        a1 = sbw.tile([P, C], F32, tag="a1")
        nc.vector.tensor_tensor(out=a1[:], in0=ps[:, 0:C],
                                in1=sneg[:, t:t + 1].to_broadcast([P, C]),
                                op=mybir.AluOpType.mult)
        nc.sync.dma_start(out=ac1_st[rs, :], in_=a1[:])
        q2 = sbw.tile([P, QT2], F32, tag="q2")
        nc.vector.memset(q2[:], 0.0)
        nc.vector.tensor_tensor(out=q2[:, 0:C], in0=ps[:, C:C2],
                                in1=sneg2[:, t:t + 1].to_broadcast([P, C]),
                                op=mybir.AluOpType.mult)
        nc.sync.dma_start(out=q2_shard[rs, :], in_=q2[:])

    for t in range(T):
        gather_tile(c_full[:], t, CT, pass3_tile, False)

    nc.gpsimd.collective_compute(
        "AllGather", mybir.AluOpType.bypass,
        replica_groups=[list(range(NCORES))],
        ins=[q2_shard[:].opt()], outs=[q2_full[:].opt()])

    # ---------------- pass 4 ----------------
    def pass4_tile(t, ps):
        rs = slice(t * P, (t + 1) * P)
        hT = sbw.tile([H, P], F32, tag="hTb")
        nc.sync.dma_start(out=hT[:], in_=hT_st[:, t * P:(t + 1) * P])
        po = paux2.tile([P, C], F32, space="PSUM", tag="pa")
        nc.tensor.matmul(out=po[:], lhsT=hT[:], rhs=vall[:, 0:C], start=True, stop=True)
        a1 = sbw.tile([P, C], F32, tag="a1b")
        nc.sync.dma_start(out=a1[:], in_=ac1_st[rs, :])
        o1 = sbw.tile([P, C], F32, tag="o1")
        nc.vector.tensor_tensor(out=o1[:], in0=ps[:, 0:C],
                                in1=sm2[:, t:t + 1].to_broadcast([P, C]),
                                op=mybir.AluOpType.mult)
        nc.vector.tensor_tensor(out=o1[:], in0=o1[:], in1=po[:], op=mybir.AluOpType.add)
        nc.vector.tensor_tensor(out=o1[:], in0=o1[:], in1=a1[:], op=mybir.AluOpType.add)
        nc.vector.tensor_tensor(out=o1[:], in0=o1[:], in1=b2_sb[:],
                                op=mybir.AluOpType.add)
        nc.sync.dma_start(out=io["out_s"][rs, :], in_=o1[:])

    for t in range(T):
        gather_tile(q2_full[:], t, QT2, pass4_tile, False)

    for p in (paux2, paux, pprop, sbw, selp, gpool, const, ident_pool):
        p.release()


# ----------------------------------------------------------------------------
# Top level
# ----------------------------------------------------------------------------

def _make_nc_and_io(cfg):
    nc = bacc.Bacc("TRN2", target_bir_lowering=False, debug=False,
                   num_devices=NCORES, num_swdge_queues=cfg.get("NQ", 4))
    F, H, C = cfg["F"], cfg["H"], cfg["C"]
    T, SLOTS, GS, NBLK = cfg["T"], cfg["SLOTS"], cfg["GS"], cfg["NBLK"]
    FT = _pad_width(F)
    QT = _pad_width(H)
    CT = _pad_width(2 * C)
    QT2 = _pad_width(C)

    def inp(name, shape, dt=F32):
        return nc.dram_tensor(name, shape, dt, kind="ExternalInput").ap()

    def internal(name, shape, dt=F32, shared=False):
        return nc.dram_tensor(name, shape, dt, kind="Internal",
                              addr_space="Shared" if shared else "Local").ap()

    io = dict(
        x_perm=inp("x_perm", [GS, FT]),
        xT_s=inp("xT_s", [FT, SLOTS]),
        idx=inp("idx", [P, NBLK * (P // 16)], I16),
        col32=inp("col32", [P, NBLK], mybir.dt.int32),
        dslot=inp("dslot", [P, NBLK]),
        discol=inp("discol", [P, NBLK]),
        sneg=inp("sneg", [P, T]),
        sdis=inp("sdis", [P, T]),
        sm2=inp("sm2", [P, T]),
        sneg2=inp("sneg2", [P, T]),
        iota=inp("iota", [P, P]),
        b1row=inp("b1row", [P, H]),
        b2row=inp("b2row", [P, C]),
        w1cat=inp("w1cat", [FT, 3 * H]),
        vall=inp("vall", [H, 3 * C]),
        out_s=nc.dram_tensor("out_s", [SLOTS, C], F32, kind="ExternalOutput").ap(),
        q_shard=internal("q_shard", [SLOTS, QT]),
        q_full=internal("q_full", [GS, QT], shared=True),
        c_shard=internal("c_shard", [SLOTS, CT]),
        c_full=internal("c_full", [GS, CT], shared=True),
        q2_shard=internal("q2_shard", [SLOTS, QT2]),
        q2_full=internal("q2_full", [GS, QT2], shared=True),
        partial1=internal("partial1", [SLOTS, H]),
        hT_st=internal("hT_st", [H, SLOTS]),
        ac1_st=internal("ac1_st", [SLOTS, C]),
    )
    return nc, io


def make_in_maps(x, W1, b1, W2, b2, meta, arrays):
    N, F = x.shape
    H = W1.shape[2]
    C = W2.shape[2]
    T, SLOTS, GS = meta["T"], meta["SLOTS"], meta["GS"]
    perm, core_of = meta["perm"], meta["core_of"]
    FT = _pad_width(F)

    x_perm = np.zeros((GS, FT), np.float32)
    x_perm[perm, :F] = np.asarray(x, np.float32)
    w1cat = np.zeros((FT, 3 * H), np.float32)
    w1cat[:F] = np.concatenate([W1[0] - W1[2], W1[1], W1[2]], axis=1)
    vall = np.concatenate([W2[0] - W2[2], W2[1], W2[2]], axis=1).astype(np.float32)
    iota = np.tile(np.arange(P, dtype=np.float32)[None, :], (P, 1))
    b1row = np.tile(np.asarray(b1, np.float32)[None, :], (P, 1))
    b2row = np.tile(np.asarray(b2, np.float32)[None, :], (P, 1))

    in_maps = []
    for c in range(NCORES):
        xs = np.zeros((SLOTS, FT), np.float32)
        nodes = np.where(core_of == c)[0]
        xs[perm[nodes] - c * SLOTS, :F] = x[nodes]
        ds = arrays["dis_slot"][c]          # [P, T]
        in_maps.append(dict(
            x_perm=x_perm,
            xT_s=np.ascontiguousarray(xs.T),
            idx=arrays["idx"][c],
            col32=arrays["col32"][c],
            dslot=arrays["dslot"][c],
            discol=arrays["discol"][c],
            sneg=-ds, sdis=ds, sm2=-2.0 * ds, sneg2=-(ds * ds),
            iota=iota, b1row=b1row, b2row=b2row,
            w1cat=w1cat, vall=vall,
        ))
    return in_maps


def kernel(x, edge_index, W1, b1, W2, b2):
    x = np.asarray(x, np.float32)
    W1 = np.asarray(W1, np.float32)
    W2 = np.asarray(W2, np.float32)
    b1 = np.asarray(b1, np.float32)
    b2 = np.asarray(b2, np.float32)
    N, F = x.shape
    H = W1.shape[2]
    C = W2.shape[2]

    gmode = os.environ.get("CHEB_GATHER", "ind")
    meta, arrays = preprocess(x, edge_index, gmode=gmode)
    cfg = dict(N=N, F=F, H=H, C=C, T=meta["T"], SLOTS=meta["SLOTS"],
               GS=meta["GS"], NBLK=meta["NBLK"], MAXB=meta["MAXB"],
               sched=meta["sched"], GMODE=gmode)

    nc, io = _make_nc_and_io(cfg)
    with tile.TileContext(nc) as tc:
        build_kernel(tc, io, cfg)
    nc.compile()

    in_maps = make_in_maps(x, W1, b1, W2, b2, meta, arrays)
    trace = bool(int(os.environ.get("CHEB_TRACE", "0")))
    if trace:
        import prof_util
        prof_util.install()
    res = bass_utils.run_bass_kernel_spmd(
        nc, in_maps, core_ids=list(range(NCORES)), trace=trace)
    flat = np.concatenate([r["out_s"] for r in res.results], axis=0)
    out = flat[meta["perm"]]
    kernel.last_results = res
    return out



# revision 22
# speedup vs baseline: 1.1332x; 1.1332x over previous
"""Trainium2 Bass kernel for 2-layer ChebConv (K=3) on a 200k-node/3.2M-edge graph.

Math (PyG ChebConv, sym norm, lambda_max=2 => L_hat = -D^-1/2 A D^-1/2):
  With dis = deg^-1/2 (0 for isolated), L z = -dis * (A (dis * z)), and A commutes
  with right-multiplication by weight matrices.  One layer (K=3):
    out = x@(W0-W2) + Tx1@W1 + 2*(-dis)*(A q),   Tx1 = -dis*(A(dis*x)),
    q   = dis*(Tx1@W2)
  Layer 2 identical with h = relu(out1 + b1), projections at width C=2 first.

Mapping: nodes are LPT-packed into per-core dest tiles of 128 slots; all node
tables are stored in one permuted layout [GS, *] replicated per core, in bf16
with the source-side dis factor baked in (so the segment-sum selector is a pure
one-hot with no per-edge weight).  Edges are bucketed by (dest tile-group,
source 32k-chunk, dest tile); each (group, chunk) bucket is gathered with ONE
Ant dma_gather call (int16 wrapped indices, 4 SWDGE queues) so the ~1us SWDGE
fixed cost amortizes over thousands of rows.  Segment-sums are one-hot-selector
matmuls in bf16, accumulated across source-chunks in SBUF f32 accumulators
(PSUM holds only one chunk's partial).  The [N,H] intermediate q and the
narrow layer-2 tables are exchanged with AllGather collectives inside one SPMD
NEFF on 8 cores; narrow tables are expanded to 256B-strided padded tables with
one strided DMA so dma_gather's 256B-elem constraint is met.  The host does
index work only.
"""
import os
import heapq
import numpy as np

import concourse.bass as bass
import concourse.bacc as bacc
import concourse.tile as tile
import concourse.bass_utils as bass_utils
from concourse import mybir
from concourse.masks import make_identity

P = 128          # partitions / edges per block / dest slots per tile
NCORES = 8
CHUNK = 32768    # int16-addressable table rows per gather call
F32 = mybir.dt.float32
BF16 = mybir.dt.bfloat16
I16 = mybir.dt.int16

XCOLS = 256      # x table row width in bf16 (512B rows: full-rate DMA)
FT = 192         # useful (padded) feature cols fed to matmuls
QCOLS = 128      # q table width bf16 (256B rows)
PADC = 128       # padded row width for the narrow layer-2 tables (256B)
NQ = 4           # SWDGE queues


# ----------------------------------------------------------------------------
# Host-side preprocessing (index work only)
# ----------------------------------------------------------------------------

def preprocess(x, edge_index, G, bpt_cap=None):
    N, F = x.shape
    row = np.asarray(edge_index[0]).astype(np.int64)
    col = np.asarray(edge_index[1]).astype(np.int64)
    E = row.shape[0]

    deg = np.bincount(row, minlength=N)
    dis = np.where(deg > 0, 1.0 / np.sqrt(np.maximum(deg, 1)), 0.0).astype(np.float32)

    npc = (N + NCORES - 1) // NCORES
    core_of = np.minimum(np.arange(N) // npc, NCORES - 1)

    avg_deg = E / max(N, 1)
    if bpt_cap is None:
        bpt_cap = max(2, int(np.ceil(avg_deg * 1.07)))
    cap = bpt_cap * P

    # --- per-core LPT packing of nodes into tiles (<=P nodes, <=cap degree) ---
    tile_of = np.zeros(N, dtype=np.int64)
    slot_of = np.zeros(N, dtype=np.int64)
    T = 0
    for c in range(NCORES):
        nodes = np.where(core_of == c)[0]
        degs = deg[nodes]
        total = int(degs.sum())
        Tc = max(int(np.ceil(len(nodes) / P)), int(np.ceil(total / (cap * 0.97))))
        while True:
            order = np.argsort(-degs, kind="stable")
            heap = [(0, 0, t) for t in range(Tc)]
            heapq.heapify(heap)
            ok = True
            tl = np.empty(len(nodes), dtype=np.int64)
            sl = np.empty(len(nodes), dtype=np.int64)
            for i in order:
                d = int(degs[i])
                spill = []
                while True:
                    if not heap:
                        ok = False
                        break
                    load, cnt, t = heapq.heappop(heap)
                    if cnt < P and load + d <= cap:
                        tl[i], sl[i] = t, cnt
                        if cnt + 1 < P:
                            heapq.heappush(heap, (load + d, cnt + 1, t))
                        break
                    elif cnt < P:
                        spill.append((load, cnt, t))
                for s in spill:
                    heapq.heappush(heap, s)
                if not ok:
                    break
            if ok:
                break
            Tc += max(1, Tc // 50)
        tile_of[nodes] = tl
        slot_of[nodes] = sl
        T = max(T, Tc)

    SLOTS = T * P
    GS = NCORES * SLOTS
    NCH = (GS + CHUNK - 1) // CHUNK
    NGRP = (T + G - 1) // G
    perm = (core_of * SLOTS + tile_of * P + slot_of).astype(np.int64)

    # --- edges sorted by (core, group, chunk, tile) ---
    colp = perm[col]
    ec = core_of[row]
    et = tile_of[row]
    eg = et // G
    ech = colp >> 15
    bkey = (ec * T + et) * NCH + ech                 # (core, tile, chunk) bucket
    skey = ((ec * NGRP + eg) * NCH + ech) * T + et   # sort order
    eorder = np.argsort(skey, kind="stable")
    colp_s = colp[eorder]
    dslot_s = slot_of[row[eorder]]
    bk_s = bkey[eorder]
    ech_s = ech[eorder]

    # bucket counts [NCORES, T, NCH]; shared schedule = max over cores
    bc = np.bincount(bkey, minlength=NCORES * T * NCH).reshape(NCORES, T, NCH)
    blocks_tc = np.ceil(bc.max(axis=0) / P).astype(np.int64)          # [T, NCH]

    # block-slot layout ordered by (group, chunk, tile)
    bs_off = np.zeros((T, NCH), dtype=np.int64)
    call_bs0 = np.zeros((NGRP, NCH), dtype=np.int64)
    call_nblk = np.zeros((NGRP, NCH), dtype=np.int64)
    pos = 0
    for g in range(NGRP):
        t0, t1 = g * G, min((g + 1) * G, T)
        for ch in range(NCH):
            call_bs0[g, ch] = pos
            for t in range(t0, t1):
                bs_off[t, ch] = pos
                pos += int(blocks_tc[t, ch])
            call_nblk[g, ch] = pos - call_bs0[g, ch]
    NBLK = pos

    # rank of each (sorted) edge within its (core, tile, chunk) bucket --
    # buckets are contiguous runs of bk_s under the (core, group, chunk, tile)
    # sort, so rank = position since the start of the current run
    first = np.ones(E, dtype=bool)
    first[1:] = bk_s[1:] != bk_s[:-1]
    run_start = np.where(first)[0]
    run_len = np.diff(np.append(run_start, E))
    rank = np.arange(E) - np.repeat(run_start, run_len)
    lane = rank % P
    blk = rank // P
    bs = bs_off[(bk_s // NCH) % T, bk_s % NCH] + blk

    # selector metadata [NCORES, P, NBLK] and wrapped-16 idx [NCORES, 128, NBLK*8]
    dslot_arr = np.full((NCORES, P, NBLK), 999.0, dtype=np.float32)
    ec_s = bk_s // (T * NCH)
    dslot_arr[ec_s, lane, bs] = dslot_s.astype(np.float32)
    posg = bs * P + lane
    idx16 = np.zeros((NCORES, 16, NBLK * (P // 16)), dtype=np.int16)
    idx16[ec_s, posg % 16, posg // 16] = (colp_s - ech_s * CHUNK).astype(np.int16)
    idx_rep = np.tile(idx16, (1, 8, 1))

    dis_slot = np.zeros((NCORES, P, T), dtype=np.float32)
    dis_slot[core_of, slot_of, tile_of] = dis

    meta = dict(N=N, F=F, E=E, T=T, SLOTS=SLOTS, GS=GS, NBLK=NBLK, NCH=NCH,
                NGRP=NGRP, G=G, blocks_tc=blocks_tc, bs_off=bs_off,
                call_bs0=call_bs0, call_nblk=call_nblk, perm=perm, dis=dis,
                core_of=core_of, pad_ratio=NBLK * P * NCORES / E - 1)
    arrays = dict(dslot=dslot_arr, idx=idx_rep, dis_slot=dis_slot)
    return meta, arrays


# ----------------------------------------------------------------------------
# Bass kernel builder (SPMD; shared schedule, per-core data)
# ----------------------------------------------------------------------------

def build_kernel(tc, io, cfg):
    nc = tc.nc
    F, H, C = cfg["F"], cfg["H"], cfg["C"]
    T, SLOTS, GS = cfg["T"], cfg["SLOTS"], cfg["GS"]
    NBLK, NCH, NGRP, G = cfg["NBLK"], cfg["NCH"], cfg["NGRP"], cfg["G"]
    blocks_tc, bs_off = cfg["blocks_tc"], cfg["bs_off"]
    call_bs0, call_nblk = cfg["call_bs0"], cfg["call_nblk"]
    C2 = 2 * C
    MAXBTC = int(blocks_tc.max())
    MAXCALL = int(call_nblk.max())
    qstate = dict(q=0)

    ident_pool = tc.alloc_tile_pool(name="ident", bufs=1)
    const = tc.alloc_tile_pool(name="const", bufs=1)
    sbw = tc.alloc_tile_pool(name="work", bufs=3)
    pprop = tc.alloc_tile_pool(name="pprop", bufs=3, space="PSUM")
    paux = tc.alloc_tile_pool(name="paux", bufs=2, space="PSUM")
    paux2 = tc.alloc_tile_pool(name="paux2", bufs=3, space="PSUM")

    ident = ident_pool.tile([P, P], BF16, tag="ident")
    make_identity(nc, ident[:])

    idx_sb = const.tile([P, NBLK * (P // 16)], I16, tag="idx")
    dslot_sb = const.tile([P, NBLK], BF16, tag="dslot")
    nc.sync.dma_start(out=idx_sb[:], in_=io["idx"][:])
    nc.sync.dma_start(out=dslot_sb[:], in_=io["dslot"][:])

    sneg = const.tile([P, T], F32, tag="sneg")
    sdis = const.tile([P, T], F32, tag="sdis")
    sm2 = const.tile([P, T], F32, tag="sm2")
    sneg2 = const.tile([P, T], F32, tag="sneg2")
    nc.sync.dma_start(out=sneg[:], in_=io["sneg"][:])
    nc.sync.dma_start(out=sdis[:], in_=io["sdis"][:])
    nc.sync.dma_start(out=sm2[:], in_=io["sm2"][:])
    nc.sync.dma_start(out=sneg2[:], in_=io["sneg2"][:])

    iota_sb = const.tile([P, P], BF16, tag="iota")
    nc.sync.dma_start(out=iota_sb[:], in_=io["iota"][:])
    b1_sb = const.tile([P, H], F32, tag="b1")
    nc.sync.dma_start(out=b1_sb[:], in_=io["b1row"][:])
    b2_sb = const.tile([P, C], F32, tag="b2")
    nc.sync.dma_start(out=b2_sb[:], in_=io["b2row"][:])

    fchunks = [(0, P), (P, FT)]
    w1A = const.tile([P, 3 * H], BF16, tag="w1A")
    nc.sync.dma_start(out=w1A[:], in_=io["w1cat"][0:P, :])
    w1B = const.tile([FT - P, 3 * H], BF16, tag="w1B")
    nc.sync.dma_start(out=w1B[:], in_=io["w1cat"][P:FT, :])
    vall = const.tile([H, 3 * C], BF16, tag="vall")
    nc.sync.dma_start(out=vall[:], in_=io["vall"][:])

    q_shard, q_full = io["q_shard"], io["q_full"]
    partial1, hT_st, ac1_st = io["partial1"], io["hT_st"], io["ac1_st"]

    def run_pass(table_ap, width, mmw, per_tile, tag, gbufs=2):
        gpool = tc.alloc_tile_pool(name=f"g{tag}", bufs=gbufs)
        selp = tc.alloc_tile_pool(name=f"s{tag}", bufs=4)
        accp = tc.alloc_tile_pool(name=f"a{tag}", bufs=2 * G)
        for g in range(NGRP):
            t0, t1 = g * G, min((g + 1) * G, T)
            acc = {}
            for ch in range(NCH):
                nb_call = int(call_nblk[g, ch])
                if nb_call == 0:
                    continue
                bs0 = int(call_bs0[g, ch])
                c0 = ch * CHUNK
                c1 = min(c0 + CHUNK, GS)
                gx = gpool.tile([P, MAXCALL * width], BF16, tag="gx")
                nc.gpsimd.dma_gather(
                    out_ap=gx[:, 0:nb_call * width].rearrange(
                        "p (k w) -> p k w", w=width),
                    in_ap=table_ap[c0:c1, :],
                    idxs_ap=idx_sb[:, bs0 * (P // 16):(bs0 + nb_call) * (P // 16)],
                    num_idxs=nb_call * P, num_idxs_reg=nb_call * P,
                    elem_size=width, queue_num=qstate["q"] % NQ,
                    single_packet=False)
                qstate["q"] += 1
                for t in range(t0, t1):
                    nbt = int(blocks_tc[t, ch])
                    if nbt == 0:
                        continue
                    tb = int(bs_off[t, ch])
                    sel = selp.tile([P, MAXBTC * P], BF16, tag="sel")
                    sel3 = sel[:, 0:nbt * P].rearrange("p (j d) -> p j d", d=P)
                    dsl = dslot_sb[:, tb:tb + nbt].unsqueeze(2).to_broadcast(
                        [P, nbt, P])
                    iot = iota_sb[:].unsqueeze(1).to_broadcast([P, nbt, P])
                    nc.vector.tensor_tensor(out=sel3, in0=dsl, in1=iot,
                                            op=mybir.AluOpType.is_equal)
                    ps = pprop.tile([P, mmw], F32, space="PSUM", tag="ps")
                    for b in range(nbt):
                        o = (tb - bs0 + b) * width
                        nc.tensor.matmul(
                            out=ps[:],
                            lhsT=sel[:, b * P:(b + 1) * P],
                            rhs=gx[:, o:o + mmw],
                            start=(b == 0), stop=(b == nbt - 1))
                    if t not in acc:
                        a = accp.tile([P, mmw], F32, tag="acc")
                        nc.scalar.activation(
                            out=a[:], in_=ps[:],
                            func=mybir.ActivationFunctionType.Copy)
                        acc[t] = a
                    else:
                        nc.vector.tensor_tensor(out=acc[t][:], in0=acc[t][:],
                                                in1=ps[:],
                                                op=mybir.AluOpType.add)
            for t in range(t0, t1):
                if t not in acc:
                    a = accp.tile([P, mmw], F32, tag="acc")
                    nc.vector.memset(a[:], 0.0)
                    acc[t] = a
                per_tile(t, acc[t])
        accp.release()
        selp.release()
        gpool.release()

    # ---------------- pass 1:  A(dis*x) -> Tx1, q, partial1 ----------------
    def pass1_tile(t, ps):
        rs = slice(t * P, (t + 1) * P)
        tx1 = sbw.tile([P, FT], BF16, tag="tx1")
        nc.scalar.activation(out=tx1[:], in_=ps[:],
                             func=mybir.ActivationFunctionType.Copy,
                             scale=sneg[:, t:t + 1])
        txT = []
        for k, (a, b) in enumerate(fchunks):
            w = b - a
            pt = paux.tile([P, P], BF16, space="PSUM", tag="ptr")
            nc.tensor.transpose(out=pt[0:w, :], in_=tx1[:, a:b], identity=ident[:])
            st = sbw.tile([P, P], BF16, tag=f"txT{k}")
            nc.scalar.activation(out=st[0:w, :], in_=pt[0:w, :],
                                 func=mybir.ActivationFunctionType.Copy)
            txT.append((st, w))
        wch = [w1A, w1B]
        pq = paux2.tile([P, H], F32, space="PSUM", tag="pa")
        for k, (st, w) in enumerate(txT):
            nc.tensor.matmul(out=pq[:], lhsT=st[0:w, :], rhs=wch[k][:, 2 * H:3 * H],
                             start=(k == 0), stop=(k == len(txT) - 1))
        qt = sbw.tile([P, QCOLS], BF16, tag="qt")
        nc.scalar.activation(out=qt[:, 0:H], in_=pq[:],
                             func=mybir.ActivationFunctionType.Copy,
                             scale=sdis[:, t:t + 1])
        nc.sync.dma_start(out=q_shard[rs, :], in_=qt[:])
        pp = paux2.tile([P, H], F32, space="PSUM", tag="pa")
        first = True
        for k, (a, b) in enumerate(fchunks):
            w = b - a
            xt = sbw.tile([P, P], BF16, tag=f"xT{k}")
            nc.sync.dma_start(out=xt[0:w, :], in_=io["xT_s"][a:b, t * P:(t + 1) * P])
            nc.tensor.matmul(out=pp[:], lhsT=xt[0:w, :], rhs=wch[k][:, 0:H],
                             start=first, stop=False)
            first = False
        for k, (st, w) in enumerate(txT):
            nc.tensor.matmul(out=pp[:], lhsT=st[0:w, :], rhs=wch[k][:, H:2 * H],
                             start=False, stop=(k == len(txT) - 1))
        p1 = sbw.tile([P, H], F32, tag="p1")
        nc.vector.tensor_tensor(out=p1[:], in0=pp[:], in1=b1_sb[:],
                                op=mybir.AluOpType.add)
        nc.sync.dma_start(out=partial1[rs, :], in_=p1[:])

    run_pass(io["x_perm"][:], XCOLS, FT, pass1_tile, "1")

    nc.gpsimd.collective_compute(
        "AllGather", mybir.AluOpType.bypass,
        replica_groups=[list(range(NCORES))],
        ins=[q_shard[:].opt()], outs=[q_full[:].opt()])

    # ---------------- pass 2:  A q -> h, c ----------------
    def pass2_tile(t, ps):
        rs = slice(t * P, (t + 1) * P)
        p1 = sbw.tile([P, H], F32, tag="p1b")
        nc.sync.dma_start(out=p1[:], in_=partial1[rs, :])
        hp = sbw.tile([P, H], F32, tag="hp")
        nc.vector.tensor_tensor(out=hp[:], in0=ps[:, 0:H],
                                in1=sm2[:, t:t + 1].to_broadcast([P, H]),
                                op=mybir.AluOpType.mult)
        ht = sbw.tile([P, H], F32, tag="ht")
        nc.vector.tensor_tensor(out=ht[:], in0=hp[:], in1=p1[:],
                                op=mybir.AluOpType.add)
        htb = sbw.tile([P, H], BF16, tag="htb")
        nc.scalar.activation(out=htb[:], in_=ht[:],
                             func=mybir.ActivationFunctionType.Relu)
        pt = paux.tile([P, P], BF16, space="PSUM", tag="ptr")
        nc.tensor.transpose(out=pt[0:H, :], in_=htb[:, 0:H], identity=ident[:])
        hT = sbw.tile([H, P], BF16, tag="hT")
        nc.scalar.activation(out=hT[:], in_=pt[0:H, :],
                             func=mybir.ActivationFunctionType.Copy)
        nc.sync.dma_start(out=hT_st[:, t * P:(t + 1) * P], in_=hT[:])
        pc = paux2.tile([P, C2], F32, space="PSUM", tag="pa")
        nc.tensor.matmul(out=pc[:], lhsT=hT[:], rhs=vall[:, C:3 * C],
                         start=True, stop=True)
        ct = sbw.tile([P, C2], BF16, tag="ct")
        nc.vector.tensor_tensor(out=ct[:], in0=pc[:],
                                in1=sdis[:, t:t + 1].to_broadcast([P, C2]),
                                op=mybir.AluOpType.mult)
        nc.sync.dma_start(out=io["c_shard"][rs, :], in_=ct[:])

    run_pass(q_full[:], QCOLS, H, pass2_tile, "2")

    nc.gpsimd.collective_compute(
        "AllGather", mybir.AluOpType.bypass,
        replica_groups=[list(range(NCORES))],
        ins=[io["c_shard"][:].opt()], outs=[io["c_full"][:].opt()])
    # expand the narrow [GS, 4] table into the 256B-strided padded table
    # (split: a DMA AP axis is a 16-bit ISA field, so < 65536 rows per copy)
    for r0 in range(0, GS, 50176):
        r1 = min(r0 + 50176, GS)
        nc.sync.dma_start(out=io["c_pad"][r0:r1, 0:C2],
                          in_=io["c_full"][r0:r1, :])

    # ---------------- pass 3:  A c ----------------
    def pass3_tile(t, ps):
        rs = slice(t * P, (t + 1) * P)
        a1 = sbw.tile([P, C], F32, tag="a1")
        nc.vector.tensor_tensor(out=a1[:], in0=ps[:, 0:C],
                                in1=sneg[:, t:t + 1].to_broadcast([P, C]),
                                op=mybir.AluOpType.mult)
        nc.sync.dma_start(out=ac1_st[rs, :], in_=a1[:])
        q2 = sbw.tile([P, C], BF16, tag="q2")
        nc.vector.tensor_tensor(out=q2[:], in0=ps[:, C:C2],
                                in1=sneg2[:, t:t + 1].to_broadcast([P, C]),
                                op=mybir.AluOpType.mult)
        nc.sync.dma_start(out=io["q2_shard"][rs, :], in_=q2[:])

    run_pass(io["c_pad"][:], PADC, C2, pass3_tile, "3")

    nc.gpsimd.collective_compute(
        "AllGather", mybir.AluOpType.bypass,
        replica_groups=[list(range(NCORES))],
        ins=[io["q2_shard"][:].opt()], outs=[io["q2_full"][:].opt()])
    for r0 in range(0, GS, 50176):
        r1 = min(r0 + 50176, GS)
        nc.sync.dma_start(out=io["q2_pad"][r0:r1, 0:C],
                          in_=io["q2_full"][r0:r1, :])

    # ---------------- pass 4:  A q2 -> out ----------------
    def pass4_tile(t, ps):
        rs = slice(t * P, (t + 1) * P)
        hT = sbw.tile([H, P], BF16, tag="hTb")
        nc.sync.dma_start(out=hT[:], in_=hT_st[:, t * P:(t + 1) * P])
        po = paux2.tile([P, C], F32, space="PSUM", tag="pa")
        nc.tensor.matmul(out=po[:], lhsT=hT[:], rhs=vall[:, 0:C],
                         start=True, stop=True)
        a1 = sbw.tile([P, C], F32, tag="a1b")
        nc.sync.dma_start(out=a1[:], in_=ac1_st[rs, :])
        o1 = sbw.tile([P, C], F32, tag="o1")
        nc.vector.tensor_tensor(out=o1[:], in0=ps[:, 0:C],
                                in1=sm2[:, t:t + 1].to_broadcast([P, C]),
                                op=mybir.AluOpType.mult)
        nc.vector.tensor_tensor(out=o1[:], in0=o1[:], in1=po[:],
                                op=mybir.AluOpType.add)
        nc.vector.tensor_tensor(out=o1[:], in0=o1[:], in1=a1[:],
                                op=mybir.AluOpType.add)
        nc.vector.tensor_tensor(out=o1[:], in0=o1[:], in1=b2_sb[:],
                                op=mybir.AluOpType.add)
        nc.sync.dma_start(out=io["out_s"][rs, :], in_=o1[:])

    run_pass(io["q2_pad"][:], PADC, C, pass4_tile, "4")

    for p in (paux2, paux, pprop, sbw, const, ident_pool):
        p.release()


# ----------------------------------------------------------------------------
# Top level
# ----------------------------------------------------------------------------

def _make_nc_and_io(cfg):
    nc = bacc.Bacc("TRN2", target_bir_lowering=False, debug=False,
                   num_devices=NCORES, num_swdge_queues=NQ)
    H, C = cfg["H"], cfg["C"]
    T, SLOTS, GS, NBLK = cfg["T"], cfg["SLOTS"], cfg["GS"], cfg["NBLK"]
    C2 = 2 * C

    def inp(name, shape, dt=F32):
        return nc.dram_tensor(name, shape, dt, kind="ExternalInput").ap()

    def internal(name, shape, dt=F32, shared=False):
        return nc.dram_tensor(name, shape, dt, kind="Internal",
                              addr_space="Shared" if shared else "Local").ap()

    io = dict(
        x_perm=inp("x_perm", [GS, XCOLS], BF16),
        xT_s=inp("xT_s", [FT, SLOTS], BF16),
        idx=inp("idx", [P, NBLK * (P // 16)], I16),
        dslot=inp("dslot", [P, NBLK], BF16),
        sneg=inp("sneg", [P, T]),
        sdis=inp("sdis", [P, T]),
        sm2=inp("sm2", [P, T]),
        sneg2=inp("sneg2", [P, T]),
        iota=inp("iota", [P, P], BF16),
        b1row=inp("b1row", [P, H]),
        b2row=inp("b2row", [P, C]),
        w1cat=inp("w1cat", [FT, 3 * H], BF16),
        vall=inp("vall", [H, 3 * C], BF16),
        out_s=nc.dram_tensor("out_s", [SLOTS, C], F32, kind="ExternalOutput").ap(),
        q_shard=internal("q_shard", [SLOTS, QCOLS], BF16),
        q_full=internal("q_full", [GS, QCOLS], BF16, shared=True),
        c_shard=internal("c_shard", [SLOTS, C2], BF16),
        c_full=internal("c_full", [GS, C2], BF16, shared=True),
        c_pad=internal("c_pad", [GS, PADC], BF16),
        q2_shard=internal("q2_shard", [SLOTS, C], BF16),
        q2_full=internal("q2_full", [GS, C], BF16, shared=True),
        q2_pad=internal("q2_pad", [GS, PADC], BF16),
        partial1=internal("partial1", [SLOTS, H]),
        hT_st=internal("hT_st", [H, SLOTS], BF16),
        ac1_st=internal("ac1_st", [SLOTS, C]),
    )
    return nc, io


def make_in_maps(x, W1, b1, W2, b2, meta, arrays):
    N, F = x.shape
    H = W1.shape[2]
    T, SLOTS, GS = meta["T"], meta["SLOTS"], meta["GS"]
    perm, core_of, dis = meta["perm"], meta["core_of"], meta["dis"]

    import ml_dtypes
    bf = ml_dtypes.bfloat16

    x_perm = np.zeros((GS, XCOLS), np.float32)
    x_perm[perm, :F] = np.asarray(x, np.float32) * dis[:, None]
    w1cat = np.zeros((FT, 3 * H), np.float32)
    w1cat[:F] = np.concatenate([W1[0] - W1[2], W1[1], W1[2]], axis=1)
    vall = np.concatenate([W2[0] - W2[2], W2[1], W2[2]], axis=1).astype(np.float32)
    iota = np.tile(np.arange(P, dtype=np.float32)[None, :], (P, 1))
    b1row = np.tile(np.asarray(b1, np.float32)[None, :], (P, 1))
    b2row = np.tile(np.asarray(b2, np.float32)[None, :], (P, 1))

    in_maps = []
    for c in range(NCORES):
        xs = np.zeros((SLOTS, FT), np.float32)
        nodes = np.where(core_of == c)[0]
        xs[perm[nodes] - c * SLOTS, :F] = x[nodes]
        ds = arrays["dis_slot"][c]          # [P, T]
        in_maps.append(dict(
            x_perm=x_perm.astype(bf),
            xT_s=np.ascontiguousarray(xs.T).astype(bf),
            idx=arrays["idx"][c],
            dslot=arrays["dslot"][c].astype(bf),
            sneg=-ds, sdis=ds, sm2=-2.0 * ds, sneg2=-(ds * ds),
            iota=iota.astype(bf), b1row=b1row, b2row=b2row,
            w1cat=w1cat.astype(bf), vall=vall.astype(bf),
        ))
    return in_maps


def kernel(x, edge_index, W1, b1, W2, b2):
    x = np.asarray(x, np.float32)
    W1 = np.asarray(W1, np.float32)
    W2 = np.asarray(W2, np.float32)
    b1 = np.asarray(b1, np.float32)
    b2 = np.asarray(b2, np.float32)
    N, F = x.shape
    H = W1.shape[2]
    C = W2.shape[2]

    G = int(os.environ.get("CHEB_G", "14"))
    meta, arrays = preprocess(x, edge_index, G)
    cfg = dict(N=N, F=F, H=H, C=C, T=meta["T"], SLOTS=meta["SLOTS"],
               GS=meta["GS"], NBLK=meta["NBLK"], NCH=meta["NCH"],
               NGRP=meta["NGRP"], G=meta["G"], blocks_tc=meta["blocks_tc"],
               bs_off=meta["bs_off"], call_bs0=meta["call_bs0"],
               call_nblk=meta["call_nblk"])

    nc, io = _make_nc_and_io(cfg)
    with tile.TileContext(nc) as tc:
        build_kernel(tc, io, cfg)
    nc.compile()

    in_maps = make_in_maps(x, W1, b1, W2, b2, meta, arrays)
    trace = bool(int(os.environ.get("CHEB_TRACE", "0")))
    if trace:
        try:
            import prof_util
            prof_util.install()
        except ImportError:
            trace = False
    res = bass_utils.run_bass_kernel_spmd(
        nc, in_maps, core_ids=list(range(NCORES)), trace=trace)
    flat = np.concatenate([r["out_s"] for r in res.results], axis=0)
    out = flat[meta["perm"]]
    kernel.last_results = res
    return out


# revision 26
# speedup vs baseline: 1.1634x; 1.0266x over previous
"""Trainium2 Bass kernel for 2-layer ChebConv (K=3) on a 200k-node/3.2M-edge graph.

Math (PyG ChebConv, sym norm, lambda_max=2 => L_hat = -D^-1/2 A D^-1/2):
  With dis = deg^-1/2 (0 for isolated), L z = -dis * (A (dis * z)), and A commutes
  with right-multiplication by weight matrices.  One layer (K=3):
    out = x@(W0-W2) + Tx1@W1 + 2*(-dis)*(A q),   Tx1 = -dis*(A(dis*x)),
    q   = dis*(Tx1@W2)
  Layer 2 identical with h = relu(out1 + b1), projections at width C=2 first.

Mapping: nodes are LPT-packed into per-core dest tiles of 128 slots; all node
tables are stored in one permuted layout [GS, *] replicated per core, in bf16
with the source-side dis factor baked in (so the segment-sum selector is a pure
one-hot with no per-edge weight).  Edges are bucketed by (dest tile-group,
source 32k-chunk, dest tile); each (group, chunk) bucket is gathered with ONE
Ant dma_gather call (int16 wrapped indices, 4 SWDGE queues) so the ~1us SWDGE
fixed cost amortizes over thousands of rows.  Segment-sums are one-hot-selector
matmuls in bf16, accumulated across source-chunks in SBUF f32 accumulators
(PSUM holds only one chunk's partial).  The [N,H] intermediate q and the
narrow layer-2 tables are exchanged with AllGather collectives inside one SPMD
NEFF on 8 cores; narrow tables are expanded to 256B-strided padded tables with
one strided DMA so dma_gather's 256B-elem constraint is met.  The host does
index work only.
"""
import os
import heapq
import numpy as np

import concourse.bass as bass
import concourse.bacc as bacc
import concourse.tile as tile
import concourse.bass_utils as bass_utils
from concourse import mybir
from concourse.masks import make_identity

P = 128          # partitions / edges per block / dest slots per tile
NCORES = 8
CHUNK = 32768    # int16-addressable table rows per gather call
F32 = mybir.dt.float32
BF16 = mybir.dt.bfloat16
I16 = mybir.dt.int16

XCOLS = 256      # x table row width in bf16 (512B rows: full-rate DMA)
FT = 192         # useful (padded) feature cols fed to matmuls
QCOLS = 128      # q table width bf16 (256B rows)
PADC = 128       # padded row width for the narrow layer-2 tables (256B)
NQ = 4           # SWDGE queues


# ----------------------------------------------------------------------------
# Host-side preprocessing (index work only)
# ----------------------------------------------------------------------------

def preprocess(x, edge_index, G, bpt_cap=None):
    N, F = x.shape
    row = np.asarray(edge_index[0]).astype(np.int64)
    col = np.asarray(edge_index[1]).astype(np.int64)
    E = row.shape[0]

    deg = np.bincount(row, minlength=N)
    dis = np.where(deg > 0, 1.0 / np.sqrt(np.maximum(deg, 1)), 0.0).astype(np.float32)

    npc = (N + NCORES - 1) // NCORES
    core_of = np.minimum(np.arange(N) // npc, NCORES - 1)

    avg_deg = E / max(N, 1)
    if bpt_cap is None:
        bpt_cap = max(2, int(np.ceil(avg_deg * 1.07)))
    cap = bpt_cap * P

    # --- per-core LPT packing of nodes into tiles (<=P nodes, <=cap degree) ---
    tile_of = np.zeros(N, dtype=np.int64)
    slot_of = np.zeros(N, dtype=np.int64)
    T = 0
    for c in range(NCORES):
        nodes = np.where(core_of == c)[0]
        degs = deg[nodes]
        total = int(degs.sum())
        Tc = max(int(np.ceil(len(nodes) / P)), int(np.ceil(total / (cap * 0.97))))
        while True:
            order = np.argsort(-degs, kind="stable")
            heap = [(0, 0, t) for t in range(Tc)]
            heapq.heapify(heap)
            ok = True
            tl = np.empty(len(nodes), dtype=np.int64)
            sl = np.empty(len(nodes), dtype=np.int64)
            for i in order:
                d = int(degs[i])
                spill = []
                while True:
                    if not heap:
                        ok = False
                        break
                    load, cnt, t = heapq.heappop(heap)
                    if cnt < P and load + d <= cap:
                        tl[i], sl[i] = t, cnt
                        if cnt + 1 < P:
                            heapq.heappush(heap, (load + d, cnt + 1, t))
                        break
                    elif cnt < P:
                        spill.append((load, cnt, t))
                for s in spill:
                    heapq.heappush(heap, s)
                if not ok:
                    break
            if ok:
                break
            Tc += max(1, Tc // 50)
        tile_of[nodes] = tl
        slot_of[nodes] = sl
        T = max(T, Tc)

    SLOTS = T * P
    GS = NCORES * SLOTS
    NCH = (GS + CHUNK - 1) // CHUNK
    NGRP = (T + G - 1) // G
    perm = (core_of * SLOTS + tile_of * P + slot_of).astype(np.int64)

    # --- edges sorted by (core, group, chunk, tile) ---
    colp = perm[col]
    ec = core_of[row]
    et = tile_of[row]
    eg = et // G
    ech = colp >> 15
    bkey = (ec * T + et) * NCH + ech                 # (core, tile, chunk) bucket
    skey = ((ec * NGRP + eg) * NCH + ech) * T + et   # sort order
    eorder = np.argsort(skey, kind="stable")
    colp_s = colp[eorder]
    dslot_s = slot_of[row[eorder]]
    bk_s = bkey[eorder]
    ech_s = ech[eorder]

    # bucket counts [NCORES, T, NCH]; shared schedule = max over cores
    bc = np.bincount(bkey, minlength=NCORES * T * NCH).reshape(NCORES, T, NCH)
    blocks_tc = np.ceil(bc.max(axis=0) / P).astype(np.int64)          # [T, NCH]

    # block-slot layout ordered by (group, chunk, tile)
    bs_off = np.zeros((T, NCH), dtype=np.int64)
    call_bs0 = np.zeros((NGRP, NCH), dtype=np.int64)
    call_nblk = np.zeros((NGRP, NCH), dtype=np.int64)
    pos = 0
    for g in range(NGRP):
        t0, t1 = g * G, min((g + 1) * G, T)
        for ch in range(NCH):
            call_bs0[g, ch] = pos
            for t in range(t0, t1):
                bs_off[t, ch] = pos
                pos += int(blocks_tc[t, ch])
            call_nblk[g, ch] = pos - call_bs0[g, ch]
    NBLK = pos

    # rank of each (sorted) edge within its (core, tile, chunk) bucket --
    # buckets are contiguous runs of bk_s under the (core, group, chunk, tile)
    # sort, so rank = position since the start of the current run
    first = np.ones(E, dtype=bool)
    first[1:] = bk_s[1:] != bk_s[:-1]
    run_start = np.where(first)[0]
    run_len = np.diff(np.append(run_start, E))
    rank = np.arange(E) - np.repeat(run_start, run_len)
    lane = rank % P
    blk = rank // P
    bs = bs_off[(bk_s // NCH) % T, bk_s % NCH] + blk

    # selector metadata [NCORES, P, NBLK] and wrapped-16 idx [NCORES, 128, NBLK*8]
    dslot_arr = np.full((NCORES, P, NBLK), 999.0, dtype=np.float32)
    ec_s = bk_s // (T * NCH)
    dslot_arr[ec_s, lane, bs] = dslot_s.astype(np.float32)
    posg = bs * P + lane
    # pad positions gather chunk-row 0 (harmless; zeroed by the 999 dslot)
    idx16 = np.zeros((NCORES, 16, NBLK * (P // 16)), dtype=np.int16)
    idx16[ec_s, posg % 16, posg // 16] = (colp_s - ech_s * CHUNK).astype(np.int16)
    idx_rep = np.tile(idx16, (1, 8, 1))

    dis_slot = np.zeros((NCORES, P, T), dtype=np.float32)
    dis_slot[core_of, slot_of, tile_of] = dis

    meta = dict(N=N, F=F, E=E, T=T, SLOTS=SLOTS, GS=GS, NBLK=NBLK, NCH=NCH,
                NGRP=NGRP, G=G, blocks_tc=blocks_tc, bs_off=bs_off,
                call_bs0=call_bs0, call_nblk=call_nblk, perm=perm, dis=dis,
                core_of=core_of, pad_ratio=NBLK * P * NCORES / E - 1)
    arrays = dict(dslot=dslot_arr, idx=idx_rep, dis_slot=dis_slot)
    return meta, arrays


# ----------------------------------------------------------------------------
# Bass kernel builder (SPMD; shared schedule, per-core data)
# ----------------------------------------------------------------------------

def build_kernel(tc, io, cfg):
    nc = tc.nc
    F, H, C = cfg["F"], cfg["H"], cfg["C"]
    T, SLOTS, GS = cfg["T"], cfg["SLOTS"], cfg["GS"]
    NBLK, NCH, NGRP, G = cfg["NBLK"], cfg["NCH"], cfg["NGRP"], cfg["G"]
    blocks_tc, bs_off = cfg["blocks_tc"], cfg["bs_off"]
    call_bs0, call_nblk = cfg["call_bs0"], cfg["call_nblk"]
    C2 = 2 * C
    MAXBTC = int(blocks_tc.max())
    MAXCALL = int(call_nblk.max())
    qstate = dict(q=0)

    ident_pool = tc.alloc_tile_pool(name="ident", bufs=1)
    const = tc.alloc_tile_pool(name="const", bufs=1)
    sbw = tc.alloc_tile_pool(name="work", bufs=3)
    pprop = tc.alloc_tile_pool(name="pprop", bufs=3, space="PSUM")
    paux = tc.alloc_tile_pool(name="paux", bufs=2, space="PSUM")
    paux2 = tc.alloc_tile_pool(name="paux2", bufs=3, space="PSUM")

    ident = ident_pool.tile([P, P], BF16, tag="ident")
    make_identity(nc, ident[:])

    idx_sb = const.tile([P, NBLK * (P // 16)], I16, tag="idx")
    dslot_sb = const.tile([P, NBLK], BF16, tag="dslot")
    nc.sync.dma_start(out=idx_sb[:], in_=io["idx"][:])
    nc.sync.dma_start(out=dslot_sb[:], in_=io["dslot"][:])

    sneg = const.tile([P, T], F32, tag="sneg")
    sdis = const.tile([P, T], F32, tag="sdis")
    sm2 = const.tile([P, T], F32, tag="sm2")
    sneg2 = const.tile([P, T], F32, tag="sneg2")
    nc.sync.dma_start(out=sneg[:], in_=io["sneg"][:])
    nc.sync.dma_start(out=sdis[:], in_=io["sdis"][:])
    nc.sync.dma_start(out=sm2[:], in_=io["sm2"][:])
    nc.sync.dma_start(out=sneg2[:], in_=io["sneg2"][:])

    iota_sb = const.tile([P, P], BF16, tag="iota")
    nc.sync.dma_start(out=iota_sb[:], in_=io["iota"][:])
    b1_sb = const.tile([P, H], F32, tag="b1")
    nc.sync.dma_start(out=b1_sb[:], in_=io["b1row"][:])
    b2_sb = const.tile([P, C], F32, tag="b2")
    nc.sync.dma_start(out=b2_sb[:], in_=io["b2row"][:])

    fchunks = [(0, P), (P, FT)]
    w1A = const.tile([P, 3 * H], BF16, tag="w1A")
    nc.sync.dma_start(out=w1A[:], in_=io["w1cat"][0:P, :])
    w1B = const.tile([FT - P, 3 * H], BF16, tag="w1B")
    nc.sync.dma_start(out=w1B[:], in_=io["w1cat"][P:FT, :])
    vall = const.tile([H, 3 * C], BF16, tag="vall")
    nc.sync.dma_start(out=vall[:], in_=io["vall"][:])

    q_shard, q_full = io["q_shard"], io["q_full"]
    partial1, hT_st, ac1_st = io["partial1"], io["hT_st"], io["ac1_st"]

    def run_pass(table_ap, width, mmw, per_tile, tag, gbufs=2):
        gpool = tc.alloc_tile_pool(name=f"g{tag}", bufs=gbufs)
        selp = tc.alloc_tile_pool(name=f"s{tag}", bufs=4)
        accp = tc.alloc_tile_pool(name=f"a{tag}", bufs=2 * G)
        fresh = [gbufs]   # memset first-use gather buffers (skipped pad lanes
                          # otherwise read uninitialized SBUF -> NaN * 0 = NaN)
        for g in range(NGRP):
            t0, t1 = g * G, min((g + 1) * G, T)
            acc = {}
            for ch in range(NCH):
                nb_call = int(call_nblk[g, ch])
                if nb_call == 0:
                    continue
                bs0 = int(call_bs0[g, ch])
                c0 = ch * CHUNK
                c1 = min(c0 + CHUNK, GS)
                gx = gpool.tile([P, MAXCALL * width], BF16, tag="gx")
                if fresh[0] > 0:
                    fresh[0] -= 1
                    nc.vector.memset(gx[:], 0.0)
                nc.gpsimd.dma_gather(
                    out_ap=gx[:, 0:nb_call * width].rearrange(
                        "p (k w) -> p k w", w=width),
                    in_ap=table_ap[c0:c1, :],
                    idxs_ap=idx_sb[:, bs0 * (P // 16):(bs0 + nb_call) * (P // 16)],
                    num_idxs=nb_call * P, num_idxs_reg=nb_call * P,
                    elem_size=width, queue_num=qstate["q"] % NQ,
                    single_packet=False)
                qstate["q"] += 1
                for t in range(t0, t1):
                    nbt = int(blocks_tc[t, ch])
                    if nbt == 0:
                        continue
                    tb = int(bs_off[t, ch])
                    sel = selp.tile([P, MAXBTC * P], BF16, tag="sel")
                    sel3 = sel[:, 0:nbt * P].rearrange("p (j d) -> p j d", d=P)
                    dsl = dslot_sb[:, tb:tb + nbt].unsqueeze(2).to_broadcast(
                        [P, nbt, P])
                    iot = iota_sb[:].unsqueeze(1).to_broadcast([P, nbt, P])
                    nc.vector.tensor_tensor(out=sel3, in0=dsl, in1=iot,
                                            op=mybir.AluOpType.is_equal)
                    ps = pprop.tile([P, mmw], F32, space="PSUM", tag="ps")
                    for b in range(nbt):
                        o = (tb - bs0 + b) * width
                        nc.tensor.matmul(
                            out=ps[:],
                            lhsT=sel[:, b * P:(b + 1) * P],
                            rhs=gx[:, o:o + mmw],
                            start=(b == 0), stop=(b == nbt - 1))
                    if t not in acc:
                        a = accp.tile([P, mmw], F32, tag="acc")
                        nc.scalar.activation(
                            out=a[:], in_=ps[:],
                            func=mybir.ActivationFunctionType.Copy)
                        acc[t] = a
                    else:
                        nc.vector.tensor_tensor(out=acc[t][:], in0=acc[t][:],
                                                in1=ps[:],
                                                op=mybir.AluOpType.add)
            for t in range(t0, t1):
                if t not in acc:
                    a = accp.tile([P, mmw], F32, tag="acc")
                    nc.vector.memset(a[:], 0.0)
                    acc[t] = a
                per_tile(t, acc[t])
        accp.release()
        selp.release()
        gpool.release()

    # ---------------- pass 1:  A(dis*x) -> Tx1, q, partial1 ----------------
    def pass1_tile(t, ps):
        rs = slice(t * P, (t + 1) * P)
        tx1 = sbw.tile([P, FT], BF16, tag="tx1")
        nc.scalar.activation(out=tx1[:], in_=ps[:],
                             func=mybir.ActivationFunctionType.Copy,
                             scale=sneg[:, t:t + 1])
        txT = []
        for k, (a, b) in enumerate(fchunks):
            w = b - a
            pt = paux.tile([P, P], BF16, space="PSUM", tag="ptr")
            nc.tensor.transpose(out=pt[0:w, :], in_=tx1[:, a:b], identity=ident[:])
            st = sbw.tile([P, P], BF16, tag=f"txT{k}")
            nc.scalar.activation(out=st[0:w, :], in_=pt[0:w, :],
                                 func=mybir.ActivationFunctionType.Copy)
            txT.append((st, w))
        wch = [w1A, w1B]
        pq = paux2.tile([P, H], F32, space="PSUM", tag="pa")
        for k, (st, w) in enumerate(txT):
            nc.tensor.matmul(out=pq[:], lhsT=st[0:w, :], rhs=wch[k][:, 2 * H:3 * H],
                             start=(k == 0), stop=(k == len(txT) - 1))
        qt = sbw.tile([P, QCOLS], BF16, tag="qt")
        nc.scalar.activation(out=qt[:, 0:H], in_=pq[:],
                             func=mybir.ActivationFunctionType.Copy,
                             scale=sdis[:, t:t + 1])
        nc.sync.dma_start(out=q_shard[rs, :], in_=qt[:])
        pp = paux2.tile([P, H], F32, space="PSUM", tag="pa")
        first = True
        for k, (a, b) in enumerate(fchunks):
            w = b - a
            xt = sbw.tile([P, P], BF16, tag=f"xT{k}")
            nc.sync.dma_start(out=xt[0:w, :], in_=io["xT_s"][a:b, t * P:(t + 1) * P])
            nc.tensor.matmul(out=pp[:], lhsT=xt[0:w, :], rhs=wch[k][:, 0:H],
                             start=first, stop=False)
            first = False
        for k, (st, w) in enumerate(txT):
            nc.tensor.matmul(out=pp[:], lhsT=st[0:w, :], rhs=wch[k][:, H:2 * H],
                             start=False, stop=(k == len(txT) - 1))
        p1 = sbw.tile([P, H], F32, tag="p1")
        nc.vector.tensor_tensor(out=p1[:], in0=pp[:], in1=b1_sb[:],
                                op=mybir.AluOpType.add)
        nc.sync.dma_start(out=partial1[rs, :], in_=p1[:])

    run_pass(io["x_perm"][:], XCOLS, FT, pass1_tile, "1")

    nc.gpsimd.collective_compute(
        "AllGather", mybir.AluOpType.bypass,
        replica_groups=[list(range(NCORES))],
        ins=[q_shard[:].opt()], outs=[q_full[:].opt()])

    # ---------------- pass 2:  A q -> h, c ----------------
    def pass2_tile(t, ps):
        rs = slice(t * P, (t + 1) * P)
        p1 = sbw.tile([P, H], F32, tag="p1b")
        nc.sync.dma_start(out=p1[:], in_=partial1[rs, :])
        hp = sbw.tile([P, H], F32, tag="hp")
        nc.vector.tensor_tensor(out=hp[:], in0=ps[:, 0:H],
                                in1=sm2[:, t:t + 1].to_broadcast([P, H]),
                                op=mybir.AluOpType.mult)
        ht = sbw.tile([P, H], F32, tag="ht")
        nc.vector.tensor_tensor(out=ht[:], in0=hp[:], in1=p1[:],
                                op=mybir.AluOpType.add)
        htb = sbw.tile([P, H], BF16, tag="htb")
        nc.scalar.activation(out=htb[:], in_=ht[:],
                             func=mybir.ActivationFunctionType.Relu)
        pt = paux.tile([P, P], BF16, space="PSUM", tag="ptr")
        nc.tensor.transpose(out=pt[0:H, :], in_=htb[:, 0:H], identity=ident[:])
        hT = sbw.tile([H, P], BF16, tag="hT")
        nc.scalar.activation(out=hT[:], in_=pt[0:H, :],
                             func=mybir.ActivationFunctionType.Copy)
        nc.sync.dma_start(out=hT_st[:, t * P:(t + 1) * P], in_=hT[:])
        pc = paux2.tile([P, C2], F32, space="PSUM", tag="pa")
        nc.tensor.matmul(out=pc[:], lhsT=hT[:], rhs=vall[:, C:3 * C],
                         start=True, stop=True)
        ct = sbw.tile([P, C2], BF16, tag="ct")
        nc.vector.tensor_tensor(out=ct[:], in0=pc[:],
                                in1=sdis[:, t:t + 1].to_broadcast([P, C2]),
                                op=mybir.AluOpType.mult)
        nc.sync.dma_start(out=io["c_shard"][rs, :], in_=ct[:])

    run_pass(q_full[:], QCOLS, H, pass2_tile, "2")

    nc.gpsimd.collective_compute(
        "AllGather", mybir.AluOpType.bypass,
        replica_groups=[list(range(NCORES))],
        ins=[io["c_shard"][:].opt()], outs=[io["c_full"][:].opt()])
    # expand the narrow [GS, 4] table into the 256B-strided padded table
    # (split: a DMA AP axis is a 16-bit ISA field, so < 65536 rows per copy)
    for r0 in range(0, GS, 50176):
        r1 = min(r0 + 50176, GS)
        nc.sync.dma_start(out=io["c_pad"][r0:r1, 0:C2],
                          in_=io["c_full"][r0:r1, :])

    # ---------------- pass 3:  A c ----------------
    def pass3_tile(t, ps):
        rs = slice(t * P, (t + 1) * P)
        a1 = sbw.tile([P, C], F32, tag="a1")
        nc.vector.tensor_tensor(out=a1[:], in0=ps[:, 0:C],
                                in1=sneg[:, t:t + 1].to_broadcast([P, C]),
                                op=mybir.AluOpType.mult)
        nc.sync.dma_start(out=ac1_st[rs, :], in_=a1[:])
        q2 = sbw.tile([P, C], BF16, tag="q2")
        nc.vector.tensor_tensor(out=q2[:], in0=ps[:, C:C2],
                                in1=sneg2[:, t:t + 1].to_broadcast([P, C]),
                                op=mybir.AluOpType.mult)
        nc.sync.dma_start(out=io["q2_shard"][rs, :], in_=q2[:])

    run_pass(io["c_pad"][:], PADC, C2, pass3_tile, "3")

    nc.gpsimd.collective_compute(
        "AllGather", mybir.AluOpType.bypass,
        replica_groups=[list(range(NCORES))],
        ins=[io["q2_shard"][:].opt()], outs=[io["q2_full"][:].opt()])
    for r0 in range(0, GS, 50176):
        r1 = min(r0 + 50176, GS)
        nc.sync.dma_start(out=io["q2_pad"][r0:r1, 0:C],
                          in_=io["q2_full"][r0:r1, :])

    # ---------------- pass 4:  A q2 -> out ----------------
    def pass4_tile(t, ps):
        rs = slice(t * P, (t + 1) * P)
        hT = sbw.tile([H, P], BF16, tag="hTb")
        nc.sync.dma_start(out=hT[:], in_=hT_st[:, t * P:(t + 1) * P])
        po = paux2.tile([P, C], F32, space="PSUM", tag="pa")
        nc.tensor.matmul(out=po[:], lhsT=hT[:], rhs=vall[:, 0:C],
                         start=True, stop=True)
        a1 = sbw.tile([P, C], F32, tag="a1b")
        nc.sync.dma_start(out=a1[:], in_=ac1_st[rs, :])
        o1 = sbw.tile([P, C], F32, tag="o1")
        nc.vector.tensor_tensor(out=o1[:], in0=ps[:, 0:C],
                                in1=sm2[:, t:t + 1].to_broadcast([P, C]),
                                op=mybir.AluOpType.mult)
        nc.vector.tensor_tensor(out=o1[:], in0=o1[:], in1=po[:],
                                op=mybir.AluOpType.add)
        nc.vector.tensor_tensor(out=o1[:], in0=o1[:], in1=a1[:],
                                op=mybir.AluOpType.add)
        nc.vector.tensor_tensor(out=o1[:], in0=o1[:], in1=b2_sb[:],
                                op=mybir.AluOpType.add)
        nc.sync.dma_start(out=io["out_s"][rs, :], in_=o1[:])

    run_pass(io["q2_pad"][:], PADC, C, pass4_tile, "4")

    for p in (paux2, paux, pprop, sbw, const, ident_pool):
        p.release()


# ----------------------------------------------------------------------------
# Top level
# ----------------------------------------------------------------------------

def _make_nc_and_io(cfg):
    nc = bacc.Bacc("TRN2", target_bir_lowering=False, debug=False,
                   num_devices=NCORES, num_swdge_queues=NQ)
    H, C = cfg["H"], cfg["C"]
    T, SLOTS, GS, NBLK = cfg["T"], cfg["SLOTS"], cfg["GS"], cfg["NBLK"]
    C2 = 2 * C

    def inp(name, shape, dt=F32):
        return nc.dram_tensor(name, shape, dt, kind="ExternalInput").ap()

    def internal(name, shape, dt=F32, shared=False):
        return nc.dram_tensor(name, shape, dt, kind="Internal",
                              addr_space="Shared" if shared else "Local").ap()

    io = dict(
        x_perm=inp("x_perm", [GS, XCOLS], BF16),
        xT_s=inp("xT_s", [FT, SLOTS], BF16),
        idx=inp("idx", [P, NBLK * (P // 16)], I16),
        dslot=inp("dslot", [P, NBLK], BF16),
        sneg=inp("sneg", [P, T]),
        sdis=inp("sdis", [P, T]),
        sm2=inp("sm2", [P, T]),
        sneg2=inp("sneg2", [P, T]),
        iota=inp("iota", [P, P], BF16),
        b1row=inp("b1row", [P, H]),
        b2row=inp("b2row", [P, C]),
        w1cat=inp("w1cat", [FT, 3 * H], BF16),
        vall=inp("vall", [H, 3 * C], BF16),
        out_s=nc.dram_tensor("out_s", [SLOTS, C], F32, kind="ExternalOutput").ap(),
        q_shard=internal("q_shard", [SLOTS, QCOLS], BF16),
        q_full=internal("q_full", [GS, QCOLS], BF16, shared=True),
        c_shard=internal("c_shard", [SLOTS, C2], BF16),
        c_full=internal("c_full", [GS, C2], BF16, shared=True),
        c_pad=internal("c_pad", [GS, PADC], BF16),
        q2_shard=internal("q2_shard", [SLOTS, C], BF16),
        q2_full=internal("q2_full", [GS, C], BF16, shared=True),
        q2_pad=internal("q2_pad", [GS, PADC], BF16),
        partial1=internal("partial1", [SLOTS, H]),
        hT_st=internal("hT_st", [H, SLOTS], BF16),
        ac1_st=internal("ac1_st", [SLOTS, C]),
    )
    return nc, io


def make_in_maps(x, W1, b1, W2, b2, meta, arrays):
    N, F = x.shape
    H = W1.shape[2]
    T, SLOTS, GS = meta["T"], meta["SLOTS"], meta["GS"]
    perm, core_of, dis = meta["perm"], meta["core_of"], meta["dis"]

    import ml_dtypes
    bf = ml_dtypes.bfloat16

    x_perm = np.zeros((GS, XCOLS), np.float32)
    x_perm[perm, :F] = np.asarray(x, np.float32) * dis[:, None]
    w1cat = np.zeros((FT, 3 * H), np.float32)
    w1cat[:F] = np.concatenate([W1[0] - W1[2], W1[1], W1[2]], axis=1)
    vall = np.concatenate([W2[0] - W2[2], W2[1], W2[2]], axis=1).astype(np.float32)
    iota = np.tile(np.arange(P, dtype=np.float32)[None, :], (P, 1))
    b1row = np.tile(np.asarray(b1, np.float32)[None, :], (P, 1))
    b2row = np.tile(np.asarray(b2, np.float32)[None, :], (P, 1))

    in_maps = []
    for c in range(NCORES):
        xs = np.zeros((SLOTS, FT), np.float32)
        nodes = np.where(core_of == c)[0]
        xs[perm[nodes] - c * SLOTS, :F] = x[nodes]
        ds = arrays["dis_slot"][c]          # [P, T]
        in_maps.append(dict(
            x_perm=x_perm.astype(bf),
            xT_s=np.ascontiguousarray(xs.T).astype(bf),
            idx=arrays["idx"][c],
            dslot=arrays["dslot"][c].astype(bf),
            sneg=-ds, sdis=ds, sm2=-2.0 * ds, sneg2=-(ds * ds),
            iota=iota.astype(bf), b1row=b1row, b2row=b2row,
            w1cat=w1cat.astype(bf), vall=vall.astype(bf),
        ))
    return in_maps


def kernel(x, edge_index, W1, b1, W2, b2):
    x = np.asarray(x, np.float32)
    W1 = np.asarray(W1, np.float32)
    W2 = np.asarray(W2, np.float32)
    b1 = np.asarray(b1, np.float32)
    b2 = np.asarray(b2, np.float32)
    N, F = x.shape
    H = W1.shape[2]
    C = W2.shape[2]

    G = int(os.environ.get("CHEB_G", "14"))
    meta, arrays = preprocess(x, edge_index, G)
    cfg = dict(N=N, F=F, H=H, C=C, T=meta["T"], SLOTS=meta["SLOTS"],
               GS=meta["GS"], NBLK=meta["NBLK"], NCH=meta["NCH"],
               NGRP=meta["NGRP"], G=meta["G"], blocks_tc=meta["blocks_tc"],
               bs_off=meta["bs_off"], call_bs0=meta["call_bs0"],
               call_nblk=meta["call_nblk"])

    nc, io = _make_nc_and_io(cfg)
    with tile.TileContext(nc) as tc:
        build_kernel(tc, io, cfg)
    nc.compile()

    in_maps = make_in_maps(x, W1, b1, W2, b2, meta, arrays)
    trace = bool(int(os.environ.get("CHEB_TRACE", "0")))
    if trace:
        try:
            import prof_util
            prof_util.install()
        except ImportError:
            trace = False
    res = bass_utils.run_bass_kernel_spmd(
        nc, in_maps, core_ids=list(range(NCORES)), trace=trace)
    flat = np.concatenate([r["out_s"] for r in res.results], axis=0)
    out = flat[meta["perm"]]
    kernel.last_results = res
    return out


# revision 29
# speedup vs baseline: 1.3194x; 1.1341x over previous
"""Trainium2 Bass kernel for 2-layer ChebConv (K=3) on a 200k-node/3.2M-edge graph.

Math (PyG ChebConv, sym norm, lambda_max=2 => L_hat = -D^-1/2 A D^-1/2):
  With dis = deg^-1/2 (0 for isolated), L z = -dis * (A (dis * z)), and A commutes
  with right-multiplication by weight matrices.  One layer (K=3):
    out = x@(W0-W2) + Tx1@W1 + 2*(-dis)*(A q),   Tx1 = -dis*(A(dis*x)),
    q   = dis*(Tx1@W2)
  Layer 2 identical with h = relu(out1 + b1), projections at width C=2 first.

Mapping: nodes are LPT-packed into per-core dest tiles of 128 slots; all node
tables are stored in one permuted layout [GS, *] replicated per core, in bf16
with the source-side dis factor baked in (so the segment-sum selector is a pure
one-hot with no per-edge weight).  Edges are bucketed by (dest tile-group,
source 32k-chunk, dest tile); each (group, chunk) bucket is gathered with ONE
Ant dma_gather call (int16 wrapped indices, 4 SWDGE queues) so the ~1us SWDGE
fixed cost amortizes over thousands of rows.  Segment-sums are one-hot-selector
matmuls in bf16, accumulated across source-chunks in SBUF f32 accumulators
(PSUM holds only one chunk's partial).  The [N,H] intermediate q and the
narrow layer-2 tables are exchanged with AllGather collectives inside one SPMD
NEFF on 8 cores; narrow tables are expanded to 256B-strided padded tables with
one strided DMA so dma_gather's 256B-elem constraint is met.  The host does
index work only.
"""
import os
import heapq
import numpy as np

import concourse.bass as bass
import concourse.bacc as bacc
import concourse.tile as tile
import concourse.bass_utils as bass_utils
from concourse import mybir
from concourse.masks import make_identity

P = 128          # partitions / edges per block / dest slots per tile
NCORES = 8
CHUNK = 32768    # int16-addressable table rows per gather call
F32 = mybir.dt.float32
BF16 = mybir.dt.bfloat16
I16 = mybir.dt.int16

XCOLS = 256      # x table row width in bf16 (512B rows: full-rate DMA)
FT = 192         # useful (padded) feature cols fed to matmuls
QCOLS = 128      # q table width bf16 (256B rows)
PADC = 128       # padded row width for the narrow layer-2 tables (256B)
NQ = 4           # SWDGE queues


# ----------------------------------------------------------------------------
# Host-side preprocessing (index work only)
# ----------------------------------------------------------------------------

def preprocess(x, edge_index, G, bpt_cap=None):
    N, F = x.shape
    row = np.asarray(edge_index[0]).astype(np.int64)
    col = np.asarray(edge_index[1]).astype(np.int64)
    E = row.shape[0]

    deg = np.bincount(row, minlength=N)
    dis = np.where(deg > 0, 1.0 / np.sqrt(np.maximum(deg, 1)), 0.0).astype(np.float32)

    npc = (N + NCORES - 1) // NCORES
    core_of = np.minimum(np.arange(N) // npc, NCORES - 1)

    avg_deg = E / max(N, 1)
    if bpt_cap is None:
        bpt_cap = max(2, int(np.ceil(avg_deg * 1.07)))
    cap = bpt_cap * P

    # --- per-core LPT packing of nodes into tiles (<=P nodes, <=cap degree) ---
    tile_of = np.zeros(N, dtype=np.int64)
    slot_of = np.zeros(N, dtype=np.int64)
    T = 0
    for c in range(NCORES):
        nodes = np.where(core_of == c)[0]
        degs = deg[nodes]
        total = int(degs.sum())
        Tc = max(int(np.ceil(len(nodes) / P)), int(np.ceil(total / (cap * 0.97))))
        while True:
            order = np.argsort(-degs, kind="stable")
            heap = [(0, 0, t) for t in range(Tc)]
            heapq.heapify(heap)
            ok = True
            tl = np.empty(len(nodes), dtype=np.int64)
            sl = np.empty(len(nodes), dtype=np.int64)
            for i in order:
                d = int(degs[i])
                spill = []
                while True:
                    if not heap:
                        ok = False
                        break
                    load, cnt, t = heapq.heappop(heap)
                    if cnt < P and load + d <= cap:
                        tl[i], sl[i] = t, cnt
                        if cnt + 1 < P:
                            heapq.heappush(heap, (load + d, cnt + 1, t))
                        break
                    elif cnt < P:
                        spill.append((load, cnt, t))
                for s in spill:
                    heapq.heappush(heap, s)
                if not ok:
                    break
            if ok:
                break
            Tc += max(1, Tc // 50)
        tile_of[nodes] = tl
        slot_of[nodes] = sl
        T = max(T, Tc)

    SLOTS = T * P
    GS = NCORES * SLOTS
    NCH = (GS + CHUNK - 1) // CHUNK
    NGRP = (T + G - 1) // G
    perm = (core_of * SLOTS + tile_of * P + slot_of).astype(np.int64)

    # --- edges sorted by (core, group, chunk, tile) ---
    colp = perm[col]
    ec = core_of[row]
    et = tile_of[row]
    eg = et // G
    ech = colp >> 15
    bkey = (ec * T + et) * NCH + ech                 # (core, tile, chunk) bucket
    skey = ((ec * NGRP + eg) * NCH + ech) * T + et   # sort order
    eorder = np.argsort(skey, kind="stable")
    colp_s = colp[eorder]
    dslot_s = slot_of[row[eorder]]
    bk_s = bkey[eorder]
    ech_s = ech[eorder]

    # bucket counts [NCORES, T, NCH]; shared schedule = max over cores
    bc = np.bincount(bkey, minlength=NCORES * T * NCH).reshape(NCORES, T, NCH)
    blocks_tc = np.ceil(bc.max(axis=0) / P).astype(np.int64)          # [T, NCH]

    # block-slot layout ordered by (group, chunk, tile)
    bs_off = np.zeros((T, NCH), dtype=np.int64)
    call_bs0 = np.zeros((NGRP, NCH), dtype=np.int64)
    call_nblk = np.zeros((NGRP, NCH), dtype=np.int64)
    pos = 0
    for g in range(NGRP):
        t0, t1 = g * G, min((g + 1) * G, T)
        for ch in range(NCH):
            call_bs0[g, ch] = pos
            for t in range(t0, t1):
                bs_off[t, ch] = pos
                pos += int(blocks_tc[t, ch])
            call_nblk[g, ch] = pos - call_bs0[g, ch]
    NBLK = pos

    # rank of each (sorted) edge within its (core, tile, chunk) bucket --
    # buckets are contiguous runs of bk_s under the (core, group, chunk, tile)
    # sort, so rank = position since the start of the current run
    first = np.ones(E, dtype=bool)
    first[1:] = bk_s[1:] != bk_s[:-1]
    run_start = np.where(first)[0]
    run_len = np.diff(np.append(run_start, E))
    rank = np.arange(E) - np.repeat(run_start, run_len)
    lane = rank % P
    blk = rank // P
    bs = bs_off[(bk_s // NCH) % T, bk_s % NCH] + blk

    # selector metadata [NCORES, P, NBLK] and wrapped-16 idx [NCORES, 128, NBLK*8]
    dslot_arr = np.full((NCORES, P, NBLK), 999.0, dtype=np.float32)
    ec_s = bk_s // (T * NCH)
    dslot_arr[ec_s, lane, bs] = dslot_s.astype(np.float32)
    posg = bs * P + lane
    # pad positions gather chunk-row 0 (harmless; zeroed by the 999 dslot).
    # NOTE: -1 skip-pads pass isolated probes but reliably crash the full
    # 8-core run (NRT INTERNAL) -- do not re-enable without a full-scale test.
    idx16 = np.zeros((NCORES, 16, NBLK * (P // 16)), dtype=np.int16)
    idx16[ec_s, posg % 16, posg // 16] = (colp_s - ech_s * CHUNK).astype(np.int16)
    idx_rep = np.tile(idx16, (1, 8, 1))

    dis_slot = np.zeros((NCORES, P, T), dtype=np.float32)
    dis_slot[core_of, slot_of, tile_of] = dis

    meta = dict(N=N, F=F, E=E, T=T, SLOTS=SLOTS, GS=GS, NBLK=NBLK, NCH=NCH,
                NGRP=NGRP, G=G, blocks_tc=blocks_tc, bs_off=bs_off,
                call_bs0=call_bs0, call_nblk=call_nblk, perm=perm, dis=dis,
                core_of=core_of, pad_ratio=NBLK * P * NCORES / E - 1)
    arrays = dict(dslot=dslot_arr, idx=idx_rep, dis_slot=dis_slot)
    return meta, arrays


# ----------------------------------------------------------------------------
# Bass kernel builder (SPMD; shared schedule, per-core data)
# ----------------------------------------------------------------------------

def build_kernel(tc, io, cfg):
    nc = tc.nc
    F, H, C = cfg["F"], cfg["H"], cfg["C"]
    T, SLOTS, GS = cfg["T"], cfg["SLOTS"], cfg["GS"]
    NBLK, NCH, NGRP, G = cfg["NBLK"], cfg["NCH"], cfg["NGRP"], cfg["G"]
    blocks_tc, bs_off = cfg["blocks_tc"], cfg["bs_off"]
    call_bs0, call_nblk = cfg["call_bs0"], cfg["call_nblk"]
    C2 = 2 * C
    MAXBTC = int(blocks_tc.max())
    MAXCALL = int(call_nblk.max())
    qstate = dict(q=0)

    ident_pool = tc.alloc_tile_pool(name="ident", bufs=1)
    const = tc.alloc_tile_pool(name="const", bufs=1)
    sbw = tc.alloc_tile_pool(name="work", bufs=3)
    pprop = tc.alloc_tile_pool(name="pprop", bufs=3, space="PSUM")
    paux = tc.alloc_tile_pool(name="paux", bufs=2, space="PSUM")
    paux2 = tc.alloc_tile_pool(name="paux2", bufs=3, space="PSUM")

    ident = ident_pool.tile([P, P], BF16, tag="ident")
    make_identity(nc, ident[:])

    idx_sb = const.tile([P, NBLK * (P // 16)], I16, tag="idx")
    dslot_sb = const.tile([P, NBLK], BF16, tag="dslot")
    nc.sync.dma_start(out=idx_sb[:], in_=io["idx"][:])
    nc.sync.dma_start(out=dslot_sb[:], in_=io["dslot"][:])

    sneg = const.tile([P, T], F32, tag="sneg")
    sdis = const.tile([P, T], F32, tag="sdis")
    sm2 = const.tile([P, T], F32, tag="sm2")
    sneg2 = const.tile([P, T], F32, tag="sneg2")
    nc.sync.dma_start(out=sneg[:], in_=io["sneg"][:])
    nc.sync.dma_start(out=sdis[:], in_=io["sdis"][:])
    nc.sync.dma_start(out=sm2[:], in_=io["sm2"][:])
    nc.sync.dma_start(out=sneg2[:], in_=io["sneg2"][:])

    iota_sb = const.tile([P, P], BF16, tag="iota")
    nc.sync.dma_start(out=iota_sb[:], in_=io["iota"][:])
    b1_sb = const.tile([P, H], F32, tag="b1")
    nc.sync.dma_start(out=b1_sb[:], in_=io["b1row"][:])
    b2_sb = const.tile([P, C], F32, tag="b2")
    nc.sync.dma_start(out=b2_sb[:], in_=io["b2row"][:])

    fchunks = [(0, P), (P, FT)]
    w1A = const.tile([P, 3 * H], BF16, tag="w1A")
    nc.sync.dma_start(out=w1A[:], in_=io["w1cat"][0:P, :])
    w1B = const.tile([FT - P, 3 * H], BF16, tag="w1B")
    nc.sync.dma_start(out=w1B[:], in_=io["w1cat"][P:FT, :])
    vall = const.tile([H, 3 * C], BF16, tag="vall")
    nc.sync.dma_start(out=vall[:], in_=io["vall"][:])

    q_shard, q_full = io["q_shard"], io["q_full"]
    partial1, hT_st, ac1_st = io["partial1"], io["hT_st"], io["ac1_st"]

    def run_pass(table_ap, width, mmw, per_tile, tag, gbufs=3):
        gpool = tc.alloc_tile_pool(name=f"g{tag}", bufs=gbufs)
        selp = tc.alloc_tile_pool(name=f"s{tag}", bufs=4)
        accp = tc.alloc_tile_pool(name=f"a{tag}", bufs=2 * G)
        fresh = [gbufs]   # memset first-use gather buffers (skipped pad lanes
                          # otherwise read uninitialized SBUF -> NaN * 0 = NaN)
        for g in range(NGRP):
            t0, t1 = g * G, min((g + 1) * G, T)
            acc = {}
            for ch in range(NCH):
                nb_call = int(call_nblk[g, ch])
                if nb_call == 0:
                    continue
                bs0 = int(call_bs0[g, ch])
                c0 = ch * CHUNK
                c1 = min(c0 + CHUNK, GS)
                gx = gpool.tile([P, MAXCALL * width], BF16, tag="gx")
                if fresh[0] > 0:
                    fresh[0] -= 1
                    nc.vector.memset(gx[:], 0.0)
                nc.gpsimd.dma_gather(
                    out_ap=gx[:, 0:nb_call * width].rearrange(
                        "p (k w) -> p k w", w=width),
                    in_ap=table_ap[c0:c1, :],
                    idxs_ap=idx_sb[:, bs0 * (P // 16):(bs0 + nb_call) * (P // 16)],
                    num_idxs=nb_call * P, num_idxs_reg=nb_call * P,
                    elem_size=width, queue_num=qstate["q"] % NQ,
                    single_packet=False)
                qstate["q"] += 1
                for t in range(t0, t1):
                    nbt = int(blocks_tc[t, ch])
                    if nbt == 0:
                        continue
                    tb = int(bs_off[t, ch])
                    sel = selp.tile([P, MAXBTC * P], BF16, tag="sel")
                    sel3 = sel[:, 0:nbt * P].rearrange("p (j d) -> p j d", d=P)
                    dsl = dslot_sb[:, tb:tb + nbt].unsqueeze(2).to_broadcast(
                        [P, nbt, P])
                    iot = iota_sb[:].unsqueeze(1).to_broadcast([P, nbt, P])
                    nc.vector.tensor_tensor(out=sel3, in0=dsl, in1=iot,
                                            op=mybir.AluOpType.is_equal)
                    ps = pprop.tile([P, mmw], F32, space="PSUM", tag="ps")
                    for b in range(nbt):
                        o = (tb - bs0 + b) * width
                        nc.tensor.matmul(
                            out=ps[:],
                            lhsT=sel[:, b * P:(b + 1) * P],
                            rhs=gx[:, o:o + mmw],
                            start=(b == 0), stop=(b == nbt - 1))
                    if t not in acc:
                        a = accp.tile([P, mmw], F32, tag="acc")
                        nc.scalar.activation(
                            out=a[:], in_=ps[:],
                            func=mybir.ActivationFunctionType.Copy)
                        acc[t] = a
                    else:
                        nc.vector.tensor_tensor(out=acc[t][:], in0=acc[t][:],
                                                in1=ps[:],
                                                op=mybir.AluOpType.add)
            for t in range(t0, t1):
                if t not in acc:
                    a = accp.tile([P, mmw], F32, tag="acc")
                    nc.vector.memset(a[:], 0.0)
                    acc[t] = a
                per_tile(t, acc[t])
        accp.release()
        selp.release()
        gpool.release()

    # ---------------- pass 1:  A(dis*x) -> Tx1, q, partial1 ----------------
    def pass1_tile(t, ps):
        rs = slice(t * P, (t + 1) * P)
        tx1 = sbw.tile([P, FT], BF16, tag="tx1")
        nc.scalar.activation(out=tx1[:], in_=ps[:],
                             func=mybir.ActivationFunctionType.Copy,
                             scale=sneg[:, t:t + 1])
        txT = []
        for k, (a, b) in enumerate(fchunks):
            w = b - a
            pt = paux.tile([P, P], BF16, space="PSUM", tag="ptr")
            nc.tensor.transpose(out=pt[0:w, :], in_=tx1[:, a:b], identity=ident[:])
            st = sbw.tile([P, P], BF16, tag=f"txT{k}")
            nc.scalar.activation(out=st[0:w, :], in_=pt[0:w, :],
                                 func=mybir.ActivationFunctionType.Copy)
            txT.append((st, w))
        wch = [w1A, w1B]
        pq = paux2.tile([P, H], F32, space="PSUM", tag="pa")
        for k, (st, w) in enumerate(txT):
            nc.tensor.matmul(out=pq[:], lhsT=st[0:w, :], rhs=wch[k][:, 2 * H:3 * H],
                             start=(k == 0), stop=(k == len(txT) - 1))
        qt = sbw.tile([P, QCOLS], BF16, tag="qt")
        nc.scalar.activation(out=qt[:, 0:H], in_=pq[:],
                             func=mybir.ActivationFunctionType.Copy,
                             scale=sdis[:, t:t + 1])
        nc.sync.dma_start(out=q_shard[rs, :], in_=qt[:])
        pp = paux2.tile([P, H], F32, space="PSUM", tag="pa")
        first = True
        for k, (a, b) in enumerate(fchunks):
            w = b - a
            xt = sbw.tile([P, P], BF16, tag=f"xT{k}")
            nc.sync.dma_start(out=xt[0:w, :], in_=io["xT_s"][a:b, t * P:(t + 1) * P])
            nc.tensor.matmul(out=pp[:], lhsT=xt[0:w, :], rhs=wch[k][:, 0:H],
                             start=first, stop=False)
            first = False
        for k, (st, w) in enumerate(txT):
            nc.tensor.matmul(out=pp[:], lhsT=st[0:w, :], rhs=wch[k][:, H:2 * H],
                             start=False, stop=(k == len(txT) - 1))
        p1 = sbw.tile([P, H], F32, tag="p1")
        nc.vector.tensor_tensor(out=p1[:], in0=pp[:], in1=b1_sb[:],
                                op=mybir.AluOpType.add)
        nc.sync.dma_start(out=partial1[rs, :], in_=p1[:])

    run_pass(io["x_perm"][:], XCOLS, FT, pass1_tile, "1")

    nc.gpsimd.collective_compute(
        "AllGather", mybir.AluOpType.bypass,
        replica_groups=[list(range(NCORES))],
        ins=[q_shard[:].opt()], outs=[q_full[:].opt()])

    # ---------------- pass 2:  A q -> h, c ----------------
    def pass2_tile(t, ps):
        rs = slice(t * P, (t + 1) * P)
        p1 = sbw.tile([P, H], F32, tag="p1b")
        nc.sync.dma_start(out=p1[:], in_=partial1[rs, :])
        hp = sbw.tile([P, H], F32, tag="hp")
        nc.vector.tensor_tensor(out=hp[:], in0=ps[:, 0:H],
                                in1=sm2[:, t:t + 1].to_broadcast([P, H]),
                                op=mybir.AluOpType.mult)
        ht = sbw.tile([P, H], F32, tag="ht")
        nc.vector.tensor_tensor(out=ht[:], in0=hp[:], in1=p1[:],
                                op=mybir.AluOpType.add)
        htb = sbw.tile([P, H], BF16, tag="htb")
        nc.scalar.activation(out=htb[:], in_=ht[:],
                             func=mybir.ActivationFunctionType.Relu)
        pt = paux.tile([P, P], BF16, space="PSUM", tag="ptr")
        nc.tensor.transpose(out=pt[0:H, :], in_=htb[:, 0:H], identity=ident[:])
        hT = sbw.tile([H, P], BF16, tag="hT")
        nc.scalar.activation(out=hT[:], in_=pt[0:H, :],
                             func=mybir.ActivationFunctionType.Copy)
        nc.sync.dma_start(out=hT_st[:, t * P:(t + 1) * P], in_=hT[:])
        pc = paux2.tile([P, C2], F32, space="PSUM", tag="pa")
        nc.tensor.matmul(out=pc[:], lhsT=hT[:], rhs=vall[:, C:3 * C],
                         start=True, stop=True)
        ct = sbw.tile([P, C2], BF16, tag="ct")
        nc.vector.tensor_tensor(out=ct[:], in0=pc[:],
                                in1=sdis[:, t:t + 1].to_broadcast([P, C2]),
                                op=mybir.AluOpType.mult)
        nc.sync.dma_start(out=io["c_shard"][rs, :], in_=ct[:])

    run_pass(q_full[:], QCOLS, H, pass2_tile, "2")

    nc.gpsimd.collective_compute(
        "AllGather", mybir.AluOpType.bypass,
        replica_groups=[list(range(NCORES))],
        ins=[io["c_shard"][:].opt()], outs=[io["c_full"][:].opt()])
    # expand the narrow [GS, 4] table into the 256B-strided padded table
    # (split: a DMA AP axis is a 16-bit ISA field, so < 65536 rows per copy)
    for r0 in range(0, GS, 50176):
        r1 = min(r0 + 50176, GS)
        nc.sync.dma_start(out=io["c_pad"][r0:r1, 0:C2],
                          in_=io["c_full"][r0:r1, :])

    # ---------------- pass 3:  A c ----------------
    def pass3_tile(t, ps):
        rs = slice(t * P, (t + 1) * P)
        a1 = sbw.tile([P, C], F32, tag="a1")
        nc.vector.tensor_tensor(out=a1[:], in0=ps[:, 0:C],
                                in1=sneg[:, t:t + 1].to_broadcast([P, C]),
                                op=mybir.AluOpType.mult)
        nc.sync.dma_start(out=ac1_st[rs, :], in_=a1[:])
        q2 = sbw.tile([P, C], BF16, tag="q2")
        nc.vector.tensor_tensor(out=q2[:], in0=ps[:, C:C2],
                                in1=sneg2[:, t:t + 1].to_broadcast([P, C]),
                                op=mybir.AluOpType.mult)
        nc.sync.dma_start(out=io["q2_shard"][rs, :], in_=q2[:])

    run_pass(io["c_pad"][:], PADC, C2, pass3_tile, "3")

    nc.gpsimd.collective_compute(
        "AllGather", mybir.AluOpType.bypass,
        replica_groups=[list(range(NCORES))],
        ins=[io["q2_shard"][:].opt()], outs=[io["q2_full"][:].opt()])
    for r0 in range(0, GS, 50176):
        r1 = min(r0 + 50176, GS)
        nc.sync.dma_start(out=io["q2_pad"][r0:r1, 0:C],
                          in_=io["q2_full"][r0:r1, :])

    # ---------------- pass 4:  A q2 -> out ----------------
    def pass4_tile(t, ps):
        rs = slice(t * P, (t + 1) * P)
        hT = sbw.tile([H, P], BF16, tag="hTb")
        nc.sync.dma_start(out=hT[:], in_=hT_st[:, t * P:(t + 1) * P])
        po = paux2.tile([P, C], F32, space="PSUM", tag="pa")
        nc.tensor.matmul(out=po[:], lhsT=hT[:], rhs=vall[:, 0:C],
                         start=True, stop=True)
        a1 = sbw.tile([P, C], F32, tag="a1b")
        nc.sync.dma_start(out=a1[:], in_=ac1_st[rs, :])
        o1 = sbw.tile([P, C], F32, tag="o1")
        nc.vector.tensor_tensor(out=o1[:], in0=ps[:, 0:C],
                                in1=sm2[:, t:t + 1].to_broadcast([P, C]),
                                op=mybir.AluOpType.mult)
        nc.vector.tensor_tensor(out=o1[:], in0=o1[:], in1=po[:],
                                op=mybir.AluOpType.add)
        nc.vector.tensor_tensor(out=o1[:], in0=o1[:], in1=a1[:],
                                op=mybir.AluOpType.add)
        nc.vector.tensor_tensor(out=o1[:], in0=o1[:], in1=b2_sb[:],
                                op=mybir.AluOpType.add)
        nc.sync.dma_start(out=io["out_s"][rs, :], in_=o1[:])

    run_pass(io["q2_pad"][:], PADC, C, pass4_tile, "4")

    for p in (paux2, paux, pprop, sbw, const, ident_pool):
        p.release()


# ----------------------------------------------------------------------------
# Top level
# ----------------------------------------------------------------------------

def _make_nc_and_io(cfg):
    nc = bacc.Bacc("TRN2", target_bir_lowering=False, debug=False,
                   num_devices=NCORES, num_swdge_queues=NQ)
    H, C = cfg["H"], cfg["C"]
    T, SLOTS, GS, NBLK = cfg["T"], cfg["SLOTS"], cfg["GS"], cfg["NBLK"]
    C2 = 2 * C

    def inp(name, shape, dt=F32):
        return nc.dram_tensor(name, shape, dt, kind="ExternalInput").ap()

    def internal(name, shape, dt=F32, shared=False):
        return nc.dram_tensor(name, shape, dt, kind="Internal",
                              addr_space="Shared" if shared else "Local").ap()

    io = dict(
        x_perm=inp("x_perm", [GS, XCOLS], BF16),
        xT_s=inp("xT_s", [FT, SLOTS], BF16),
        idx=inp("idx", [P, NBLK * (P // 16)], I16),
        dslot=inp("dslot", [P, NBLK], BF16),
        sneg=inp("sneg", [P, T]),
        sdis=inp("sdis", [P, T]),
        sm2=inp("sm2", [P, T]),
        sneg2=inp("sneg2", [P, T]),
        iota=inp("iota", [P, P], BF16),
        b1row=inp("b1row", [P, H]),
        b2row=inp("b2row", [P, C]),
        w1cat=inp("w1cat", [FT, 3 * H], BF16),
        vall=inp("vall", [H, 3 * C], BF16),
        out_s=nc.dram_tensor("out_s", [SLOTS, C], F32, kind="ExternalOutput").ap(),
        q_shard=internal("q_shard", [SLOTS, QCOLS], BF16),
        q_full=internal("q_full", [GS, QCOLS], BF16, shared=True),
        c_shard=internal("c_shard", [SLOTS, C2], BF16),
        c_full=internal("c_full", [GS, C2], BF16, shared=True),
        c_pad=internal("c_pad", [GS, PADC], BF16),
        q2_shard=internal("q2_shard", [SLOTS, C], BF16),
        q2_full=internal("q2_full", [GS, C], BF16, shared=True),
        q2_pad=internal("q2_pad", [GS, PADC], BF16),
        partial1=internal("partial1", [SLOTS, H]),
        hT_st=internal("hT_st", [H, SLOTS], BF16),
        ac1_st=internal("ac1_st", [SLOTS, C]),
    )
    return nc, io


def make_in_maps(x, W1, b1, W2, b2, meta, arrays):
    N, F = x.shape
    H = W1.shape[2]
    T, SLOTS, GS = meta["T"], meta["SLOTS"], meta["GS"]
    perm, core_of, dis = meta["perm"], meta["core_of"], meta["dis"]

    import ml_dtypes
    bf = ml_dtypes.bfloat16

    x_perm = np.zeros((GS, XCOLS), np.float32)
    x_perm[perm, :F] = np.asarray(x, np.float32) * dis[:, None]
    w1cat = np.zeros((FT, 3 * H), np.float32)
    w1cat[:F] = np.concatenate([W1[0] - W1[2], W1[1], W1[2]], axis=1)
    vall = np.concatenate([W2[0] - W2[2], W2[1], W2[2]], axis=1).astype(np.float32)
    iota = np.tile(np.arange(P, dtype=np.float32)[None, :], (P, 1))
    b1row = np.tile(np.asarray(b1, np.float32)[None, :], (P, 1))
    b2row = np.tile(np.asarray(b2, np.float32)[None, :], (P, 1))

    in_maps = []
    for c in range(NCORES):
        xs = np.zeros((SLOTS, FT), np.float32)
        nodes = np.where(core_of == c)[0]
        xs[perm[nodes] - c * SLOTS, :F] = x[nodes]
        ds = arrays["dis_slot"][c]          # [P, T]
        in_maps.append(dict(
            x_perm=x_perm.astype(bf),
            xT_s=np.ascontiguousarray(xs.T).astype(bf),
            idx=arrays["idx"][c],
            dslot=arrays["dslot"][c].astype(bf),
            sneg=-ds, sdis=ds, sm2=-2.0 * ds, sneg2=-(ds * ds),
            iota=iota.astype(bf), b1row=b1row, b2row=b2row,
            w1cat=w1cat.astype(bf), vall=vall.astype(bf),
        ))
    return in_maps


def kernel(x, edge_index, W1, b1, W2, b2):
    x = np.asarray(x, np.float32)
    W1 = np.asarray(W1, np.float32)
    W2 = np.asarray(W2, np.float32)
    b1 = np.asarray(b1, np.float32)
    b2 = np.asarray(b2, np.float32)
    N, F = x.shape
    H = W1.shape[2]
    C = W2.shape[2]

    G = int(os.environ.get("CHEB_G", "14"))
    meta, arrays = preprocess(x, edge_index, G)
    cfg = dict(N=N, F=F, H=H, C=C, T=meta["T"], SLOTS=meta["SLOTS"],
               GS=meta["GS"], NBLK=meta["NBLK"], NCH=meta["NCH"],
               NGRP=meta["NGRP"], G=meta["G"], blocks_tc=meta["blocks_tc"],
               bs_off=meta["bs_off"], call_bs0=meta["call_bs0"],
               call_nblk=meta["call_nblk"])

    nc, io = _make_nc_and_io(cfg)
    with tile.TileContext(nc) as tc:
        build_kernel(tc, io, cfg)
    nc.compile()

    in_maps = make_in_maps(x, W1, b1, W2, b2, meta, arrays)
    trace = bool(int(os.environ.get("CHEB_TRACE", "0")))
    if trace:
        try:
            import prof_util
            prof_util.install()
        except ImportError:
            trace = False
    res = bass_utils.run_bass_kernel_spmd(
        nc, in_maps, core_ids=list(range(NCORES)), trace=trace)
    flat = np.concatenate([r["out_s"] for r in res.results], axis=0)
    out = flat[meta["perm"]]
    kernel.last_results = res
    return out
